# revision 1
# baseline (speedup 1.0000x reference)
"""Trainium2 Bass kernel for 3-layer GAT + BN/ReLU + global max pool + FC.

Sharding: nodes renumbered so each of the 16 graphs (batch segments) gets a
fixed padded block of SEGLEN nodes; core k owns segments {2k,2k+1} (NLOC
contiguous device nodes). Edges partitioned by destination core, grouped in
128-dst windows; segmented softmax-sum + feature aggregation are done as PE
matmuls against on-device-built 0/1 selector matrices. Per-edge source rows
come from one dma_gather per window out of a per-layer DRAM table
[h bf16 | als fp32 | ald fp32 | pad]. Params replicated; one AllGather per
layer carries channel-major pre-BN aggregated output + BN partial stats.
"""
import sys

sys.path.insert(0, "/opt/trn_rl_repo")
sys.path.insert(0, "/opt/trn_rl_repo/concourse")

import numpy as np
import ml_dtypes

import concourse.bass as bass
import concourse.tile as tile
import concourse.mybir as mybir
from concourse import bacc
from concourse.bass import IndirectOffsetOnAxis

P = 128
NCORES = 8
H = 8
EPS_BN = 1e-5
NEG_SLOPE = 0.2
F32 = mybir.dt.float32
BF16 = mybir.dt.bfloat16

CINS = [3, 128, 256]
CS = [16, 32, 64]
HCS = [128, 256, 512]
ROWES = [256, 384, 640]  # table row width in bf16 elems (256B multiples)
NTILES = [1, 2, 4]       # HC/128 per layer
NKS = [1, 1, 2]          # K-tiles per dense matmul

AX = mybir.AxisListType
ALU = mybir.AluOpType
ACTF = mybir.ActivationFunctionType


def apx(sl, dims):
    """AP with custom free dims [(step, count), ...] on a tile slice's base."""
    return bass.AP(sl.tensor, sl.offset,
                   [list(sl.ap[0])] + [list(d) for d in dims])


def hoist_excess_waits(nc, cap=1):
    cnt = [0]
    total = 0
    for f in nc.m.functions:
        for b in f.blocks:
            if not any(
                i.sync_info is not None and i.sync_info.on_wait
                and len(i.sync_info.on_wait) > cap
                for i in b.instructions
            ):
                continue
            new_insts = []
            for inst in b.instructions:
                si = inst.sync_info
                if si is not None and si.on_wait and len(si.on_wait) > cap:
                    waits = list(si.on_wait)
                    for w in waits[:-cap]:
                        cnt[0] += 1
                        new_insts.append(
                            mybir.InstNoOp(
                                name=f"hw-{cnt[0]}",
                                engine=inst.engine,
                                ins=[],
                                outs=[],
                                sync_info=mybir.SyncInfo(on_wait=[w],
                                                         on_update=[]),
                            )
                        )
                        total += 1
                    inst.sync_info = mybir.SyncInfo(
                        on_wait=waits[-cap:], on_update=list(si.on_update))
                new_insts.append(inst)
            b.instructions = new_insts
    return total


def patch_walrus():
    from concourse import bass_utils as bu
    if getattr(bu, "_dge_patched", False):
        return
    orig = bu.get_walrus_args

    def patched(*a, **k):
        return orig(*a, **k) + [
            "--dge-levels=io,spill_reload,scalar_dynamic_offset,"
            "vector_dynamic_offsets,dst_reduce"
        ]

    bu.get_walrus_args = patched
    bu._dge_patched = True


def build_gnn(SEGLEN, NSUB, n_real, stages=99, repeat=1):
    nsubs = list(NSUB) if isinstance(NSUB, (list, tuple)) else None
    NLOC = 2 * SEGLEN
    NW = NLOC // P
    NPD = 16 * SEGLEN
    SEGW = NW // 2
    if nsubs is None:
        nsubs = [NSUB] * NW
    csum = [0]
    for v in nsubs:
        csum.append(csum[-1] + v)
    TS = csum[-1]  # total subchunks per core

    nc = bacc.Bacc("TRN2", target_bir_lowering=False, debug=False,
                   num_devices=NCORES, num_swdge_queues=2)

    # ---------------- I/O ----------------
    xT = nc.dram_tensor("xT", [3, NPD], BF16, kind="ExternalInput")
    wc = [nc.dram_tensor(f"wc{l}", [P if l > 1 else 3,
                                    NKS[l - 1] * (HCS[l - 1] + 16)], BF16,
                         kind="ExternalInput") for l in (1, 2, 3)]
    bng = [nc.dram_tensor(f"bng{l}", [P, NTILES[l - 1]], F32,
                          kind="ExternalInput") for l in (1, 2, 3)]
    bnb = [nc.dram_tensor(f"bnb{l}", [P, NTILES[l - 1]], F32,
                          kind="ExternalInput") for l in (1, 2, 3)]
    fcwb = nc.dram_tensor("fcwb", [513, 10], F32, kind="ExternalInput")
    srcs = nc.dram_tensor("srcs", [P, TS * 8], mybir.dt.int16,
                          kind="ExternalInput")
    dstloc = nc.dram_tensor("dstloc", [P, TS], BF16,
                            kind="ExternalInput")
    dstrep = nc.dram_tensor("dstrep", [P, TS * P], BF16,
                            kind="ExternalInput")
    dstids = nc.dram_tensor("dstids", [P, NW], mybir.dt.int32,
                            kind="ExternalInput")
    pmask = nc.dram_tensor("pmask", [1, NLOC], F32, kind="ExternalInput")
    ioro = nc.dram_tensor("ioro", [P, P], BF16, kind="ExternalInput")
    ioco = nc.dram_tensor("ioco", [P, 1], BF16, kind="ExternalInput")
    idnt = nc.dram_tensor("idnt", [P, P], F32, kind="ExternalInput")
    out = nc.dram_tensor("out", [16, 10], F32, kind="ExternalOutput")

    # ---------------- internal DRAM ----------------
    tables = [nc.dram_tensor(f"table{l}", [NPD, ROWES[l - 1]], BF16)
              for l in (1, 2, 3)]
    ag_in = [nc.dram_tensor(f"ag{l}_in", [NTILES[l - 1] * P, NLOC + 4], BF16)
             for l in (1, 2)]
    ag_out = [nc.dram_tensor(f"ag{l}_out",
                             [NCORES * NTILES[l - 1] * P, NLOC + 4], BF16)
              for l in (1, 2)]
    st3_in = nc.dram_tensor("st3_in", [4 * P, 4], BF16)
    st3_out = nc.dram_tensor("st3_out", [NCORES * 4 * P, 4], BF16)
    pl_in = nc.dram_tensor("pl_in", [4 * P, 2], F32)
    pl_out = nc.dram_tensor("pl_out", [NCORES * 4 * P, 2], F32)

    RG = [list(range(NCORES))]

    with tile.TileContext(nc) as tc, tc.tile_pool(name="const", bufs=1) as cp:
        # ---------- constants ----------
        iota_row = cp.tile([P, P], BF16, tag="ioro", name="ioro")
        nc.sync.dma_start(iota_row[:], ioro.ap())
        iota_col = cp.tile([P, 1], BF16, tag="ioco", name="ioco")
        nc.sync.dma_start(iota_col[:], ioco.ap())
        ident = cp.tile([P, P], F32, tag="idnt", name="idnt")
        nc.sync.dma_start(ident[:], idnt.ap())
        onesf = cp.tile([1, P], F32, tag="onesf", name="onesf")
        nc.vector.memset(onesf[:], 1.0)
        ones16 = cp.tile([1, 16], F32, tag="ones16", name="ones16")
        nc.vector.memset(ones16[:], 1.0)

        xT_t = cp.tile([3, NPD], BF16, tag="xT", name="xT")
        nc.sync.dma_start(xT_t[:], xT.ap())
        wc_t = []
        for i in range(3):
            t = cp.tile([P if i > 0 else 3, NKS[i] * (HCS[i] + 16)], BF16,
                        tag=f"wc{i}", name=f"wc{i}")
            nc.sync.dma_start(t[:], wc[i].ap())
            wc_t.append(t)
        bng_t, bnb_t = [], []
        for i in range(3):
            tg = cp.tile([P, NTILES[i]], F32, tag=f"bng{i}", name=f"bng{i}")
            tb = cp.tile([P, NTILES[i]], F32, tag=f"bnb{i}", name=f"bnb{i}")
            nc.sync.dma_start(tg[:], bng[i].ap())
            nc.sync.dma_start(tb[:], bnb[i].ap())
            bng_t.append(tg)
            bnb_t.append(tb)
        fcw_t = []
        for c in range(4):
            t = cp.tile([P, 10], F32, tag=f"fcw{c}", name=f"fcw{c}")
            nc.sync.dma_start(t[:], fcwb.ap()[c * P:(c + 1) * P, :])
            fcw_t.append(t)
        fcb_t = cp.tile([1, 10], F32, tag="fcb", name="fcb")
        nc.sync.dma_start(fcb_t[:], fcwb.ap()[512:513, :])
        pmask_t = cp.tile([1, NLOC], F32, tag="pmask", name="pmask")
        nc.sync.dma_start(pmask_t[:], pmask.ap())
        idx_t = cp.tile([P, TS * 8], mybir.dt.int16, tag="idx", name="idx")
        nc.sync.dma_start(idx_t[:], srcs.ap())
        dloc_t = cp.tile([P, TS], BF16, tag="dloc", name="dloc")
        nc.sync.dma_start(dloc_t[:], dstloc.ap())
        dids_t = cp.tile([P, NW], mybir.dt.int32, tag="dids", name="dids")
        nc.sync.dma_start(dids_t[:], dstids.ap())

        def compute_AB(pool, s12, gi, c):
            mu = pool.tile([P, 1], F32, tag="mu", name="mu")
            nc.vector.tensor_scalar(mu[:], s12[:, 0:1], 1.0 / n_real, None,
                                    op0=ALU.mult)
            ex2 = pool.tile([P, 1], F32, tag="ex2", name="ex2")
            nc.vector.tensor_scalar(ex2[:], s12[:, 1:2], 1.0 / n_real, None,
                                    op0=ALU.mult)
            var = pool.tile([P, 1], F32, tag="var", name="var")
            nc.vector.tensor_tensor(out=var[:], in0=mu[:], in1=mu[:],
                                    op=ALU.mult)
            nc.vector.tensor_tensor(out=var[:], in0=ex2[:], in1=var[:],
                                    op=ALU.subtract)
            nc.vector.tensor_scalar(var[:], var[:], EPS_BN, None, op0=ALU.add)
            sd = pool.tile([P, 1], F32, tag="sd", name="sd")
            nc.scalar.sqrt(sd[:], var[:])
            rs = pool.tile([P, 1], F32, tag="rs", name="rs")
            nc.vector.reciprocal(rs[:], sd[:])
            A = pool.tile([P, 1], F32, tag="A", name="A")
            nc.vector.tensor_tensor(out=A[:], in0=rs[:],
                                    in1=bng_t[gi][:, c:c + 1], op=ALU.mult)
            B = pool.tile([P, 1], F32, tag="B", name="B")
            nc.vector.tensor_tensor(out=B[:], in0=mu[:], in1=A[:],
                                    op=ALU.mult)
            nc.vector.tensor_tensor(out=B[:], in0=bnb_t[gi][:, c:c + 1],
                                    in1=B[:], op=ALU.subtract)
            return A, B

        def dense_phase(l, x_tiles):
            li = l - 1
            HC, ROWE = HCS[li], ROWES[li]
            nk = NKS[li]
            WCW = HC + 16
            with (
                tc.tile_pool(name=f"d{l}", bufs=4) as dp,
                tc.tile_pool(name=f"dps{l}", bufs=3, space="PSUM") as pp,
            ):
                fused = (HC + 16) <= 512
                for k in range(NCORES):
                    for jj in range(NW):
                        psA = pp.tile([P, HC + 16 if fused else HC], F32,
                                      tag="psA", name="psA")
                        psB = None if fused else pp.tile([P, 16], F32,
                                                         tag="psB", name="psB")
                        for c in range(nk):
                            if l == 1:
                                lhs = xT_t[:, k * NLOC + jj * P:
                                           k * NLOC + jj * P + P]
                            else:
                                lhs = x_tiles[(k, c)][:, jj * P:(jj + 1) * P]
                            if fused:
                                nc.tensor.matmul(
                                    out=psA[:], lhsT=lhs,
                                    rhs=wc_t[li][:, c * WCW:(c + 1) * WCW],
                                    start=(c == 0), stop=(c == nk - 1))
                            else:
                                rhsW = wc_t[li][:, c * WCW:c * WCW + HC]
                                rhsb = wc_t[li][:, c * WCW + HC:(c + 1) * WCW]
                                nc.tensor.matmul(out=psA[:], lhsT=lhs,
                                                 rhs=rhsW, start=(c == 0),
                                                 stop=(c == nk - 1))
                                nc.tensor.matmul(out=psB[:], lhsT=lhs,
                                                 rhs=rhsb, start=(c == 0),
                                                 stop=(c == nk - 1))
                        row = dp.tile([P, ROWE], BF16, tag="row", name="row")
                        if fused:
                            nc.vector.tensor_copy(row[:, 0:HC + 16], psA[:])
                        else:
                            nc.vector.tensor_copy(row[:, 0:HC], psA[:])
                            nc.vector.tensor_copy(row[:, HC:HC + 16], psB[:])
                        nc.vector.memset(row[:, HC + 16:ROWE], 0)
                        base = k * NLOC + jj * P
                        nc.sync.dma_start(tables[li].ap()[base:base + P, :],
                                          row[:])

        def edge_phase(l):
            li = l - 1
            HC, C, ROWE = HCS[li], CS[li], ROWES[li]
            ntile = NTILES[li]
            with (
                tc.tile_pool(name=f"e{l}", bufs=3) as ep,
                tc.tile_pool(name=f"eg{l}", bufs=3) as gp,
                tc.tile_pool(name=f"ez{l}", bufs=1) as zp,
                tc.tile_pool(name=f"eps{l}", bufs=2, space="PSUM") as pp,
                tc.tile_pool(name=f"epf{l}", bufs=2, space="PSUM") as ppf,
            ):
                if l < 3:
                    zT = [zp.tile([P, NLOC + 4], BF16, tag=f"zT{c}", name=f"zT{c}")
                          for c in range(ntile)]
                else:
                    s12 = [zp.tile([P, 2], F32, tag=f"s12{c}", name=f"s12{c}")
                           for c in range(ntile)]
                    sgm = [zp.tile([P, 2], F32, tag=f"sgm{c}", name=f"sgm{c}")
                           for c in range(ntile)]
                    for c in range(ntile):
                        nc.vector.memset(s12[c][:], 0.0)
                        nc.vector.memset(sgm[c][:], -3e38)

                for w in range(NW):
                    ns = nsubs[w]
                    base = csum[w]
                    g = gp.tile([P, max(nsubs), ROWE], BF16, tag="g",
                                name="g")[:, 0:ns, :]
                    nc.gpsimd.dma_gather(
                        out_ap=g[:],
                        in_ap=tables[li].ap(),
                        idxs_ap=idx_t[:, base * 8:(base + ns) * 8],
                        num_idxs=ns * P,
                        num_idxs_reg=ns * P,
                        elem_size=ROWE,
                        single_packet=False,
                        queue_num=w % 2,
                    )
                    aldbf = ep.tile([P, 8], BF16, tag="aldbf", name="aldbf")
                    nc.gpsimd.indirect_dma_start(
                        out=aldbf[:],
                        out_offset=None,
                        in_=tables[li].ap(),
                        in_offset=IndirectOffsetOnAxis(
                            ap=dids_t[:, w:w + 1], axis=0),
                        element_offset=HC + 8,
                    )

                    sel = ep.tile([P, max(nsubs), P], BF16, tag="sel",
                                  name="sel")[:, 0:ns, :]
                    dsl = dloc_t[:, base:base + ns]
                    nc.vector.tensor_tensor(
                        out=sel[:],
                        in0=apx(dsl, [(1, ns), (0, P)]),
                        in1=apx(iota_row[:], [(0, ns), (1, P)]),
                        op=ALU.is_equal)
                    selT = ep.tile([P, max(nsubs), P], BF16, tag="selT",
                                   name="selT")[:, 0:ns, :]
                    drep = ep.tile([P, max(nsubs) * P], BF16, tag="drep",
                                   name="drep")[:, 0:ns * P]
                    nc.sync.dma_start(
                        drep[:],
                        dstrep.ap()[:, base * P:(base + ns) * P])
                    nc.vector.tensor_tensor(
                        out=selT[:],
                        in0=apx(iota_col[:], [(0, ns), (0, P)]),
                        in1=apx(drep[:], [(P, ns), (1, P)]),
                        op=ALU.is_equal)

                    psew = pp.tile([P, max(nsubs) * 8], F32, tag="psew",
                                   name="psew")[:, 0:ns * 8]
                    for s in range(ns):
                        nc.tensor.matmul(out=psew[:, s * 8:(s + 1) * 8],
                                         lhsT=selT[:, s, :], rhs=aldbf[:],
                                         start=True, stop=True)
                    ew = ep.tile([P, max(nsubs), 8], F32, tag="ew",
                                 name="ew")[:, 0:ns, :]
                    nc.vector.tensor_tensor(
                        out=ew[:],
                        in0=apx(g[:, 0, HC:HC + 8], [(ROWE, ns), (1, 8)]),
                        in1=apx(psew[:], [(8, ns), (1, 8)]),
                        op=ALU.add)
                    ew2 = ep.tile([P, max(nsubs), 8], F32, tag="ew2",
                                  name="ew2")[:, 0:ns, :]
                    nc.vector.tensor_scalar(ew2[:], ew[:], NEG_SLOPE, None,
                                            op0=ALU.mult)
                    nc.vector.tensor_tensor(out=ew2[:], in0=ew[:], in1=ew2[:],
                                            op=ALU.max)
                    ewx = ep.tile([P, max(nsubs), 8], F32, tag="ewx",
                                  name="ewx")[:, 0:ns, :]
                    nc.scalar.activation(ewx[:], ew2[:], ACTF.Exp)
                    wbf = ep.tile([P, max(nsubs), 8], BF16, tag="wbf",
                                  name="wbf")[:, 0:ns, :]
                    nc.scalar.copy(wbf[:], ewx[:])

                    psf = ppf.tile([P, HC], F32, tag="psf", name="psf")
                    psd = pp.tile([P, 8], F32, tag="psd", name="psd")
                    for s in range(ns):
                        wh = ep.tile([P, H, C], BF16, tag="wh", name="wh")
                        gsl = g[:, s, 0:HC]
                        wsl = wbf[:, s, :]
                        nc.vector.tensor_tensor(
                            out=wh[:],
                            in0=apx(gsl, [(C, H), (1, C)]),
                            in1=apx(wsl, [(1, H), (0, C)]),
                            op=ALU.mult)
                        nc.tensor.matmul(
                            out=psf[:], lhsT=sel[:, s, :],
                            rhs=wh[:].rearrange("p h c -> p (h c)"),
                            start=(s == 0), stop=(s == ns - 1))
                        nc.tensor.matmul(
                            out=psd[:], lhsT=sel[:, s, :], rhs=wsl,
                            start=(s == 0), stop=(s == ns - 1))

                    den = ep.tile([P, 8], F32, tag="den", name="den")
                    nc.vector.tensor_scalar(den[:], psd[:], 1e-16, None,
                                            op0=ALU.add)
                    rec = ep.tile([P, 8], F32, tag="rec", name="rec")
                    nc.vector.reciprocal(rec[:], den[:])
                    z = ep.tile([P, HC], F32, tag="z", name="z")
                    for hd in range(H):
                        nc.scalar.mul(z[:, hd * C:(hd + 1) * C],
                                      psf[:, hd * C:(hd + 1) * C],
                                      rec[:, hd:hd + 1])

                    for c in range(ntile):
                        pt = pp.tile([P, P], F32, tag="pt", name="pt")
                        nc.tensor.transpose(pt[:], z[:, c * P:(c + 1) * P],
                                            ident[:])
                        if l < 3:
                            nc.scalar.copy(
                                zT[c][:, w * P:(w + 1) * P], pt[:])
                        else:
                            s1w = ep.tile([P, 1], F32, tag="s1w", name="s1w")
                            nc.vector.reduce_sum(s1w[:], pt[:], axis=AX.X)
                            nc.vector.tensor_tensor(
                                out=s12[c][:, 0:1], in0=s12[c][:, 0:1],
                                in1=s1w[:], op=ALU.add)
                            sq = ep.tile([P, P], F32, tag="sq", name="sq")
                            s2w = ep.tile([P, 1], F32, tag="s2w", name="s2w")
                            nc.scalar.activation(sq[:], pt[:], ACTF.Square,
                                                 accum_out=s2w[:])
                            nc.vector.tensor_tensor(
                                out=s12[c][:, 1:2], in0=s12[c][:, 1:2],
                                in1=s2w[:], op=ALU.add)
                            nc.tensor.matmul(
                                out=pt[:], lhsT=onesf[:],
                                rhs=pmask_t[:, w * P:(w + 1) * P],
                                start=False, stop=True, skip_group_check=True)
                            wmax = ep.tile([P, 1], F32, tag="wmax", name="wmax")
                            nc.vector.reduce_max(wmax[:], pt[:], axis=AX.X)
                            seg = 0 if w < SEGW else 1
                            nc.vector.tensor_tensor(
                                out=sgm[c][:, seg:seg + 1],
                                in0=sgm[c][:, seg:seg + 1], in1=wmax[:],
                                op=ALU.max)

                if l < 3:
                    for c in range(ntile):
                        s1 = ep.tile([P, 1], F32, tag="s1", name="s1")
                        nc.vector.reduce_sum(s1[:], zT[c][:, 0:NLOC],
                                             axis=AX.X)
                        scr = ep.tile([P, NLOC], BF16, tag="scr", name="scr")
                        s2 = ep.tile([P, 1], F32, tag="s2", name="s2")
                        nc.scalar.activation(scr[:], zT[c][:, 0:NLOC],
                                             ACTF.Square, accum_out=s2[:])
                        s12t = ep.tile([P, 2], F32, tag="s12t", name="s12t")
                        nc.vector.tensor_copy(s12t[:, 0:1], s1[:])
                        nc.vector.tensor_copy(s12t[:, 1:2], s2[:])
                        # pack fp32 sums as bf16 hi/lo pairs (finite in bf16)
                        nc.vector.tensor_copy(zT[c][:, NLOC:NLOC + 2],
                                              s12t[:])
                        hif = ep.tile([P, 2], F32, tag="hif", name="hif")
                        nc.vector.tensor_copy(hif[:],
                                              zT[c][:, NLOC:NLOC + 2])
                        lo = ep.tile([P, 2], F32, tag="lo", name="lo")
                        nc.vector.tensor_tensor(out=lo[:], in0=s12t[:],
                                                in1=hif[:], op=ALU.subtract)
                        nc.vector.tensor_copy(zT[c][:, NLOC + 2:NLOC + 4],
                                              lo[:])
                        nc.sync.dma_start(
                            ag_in[li].ap()[c * P:(c + 1) * P, :], zT[c][:])
                    nc.gpsimd.collective_compute(
                        "AllGather", ALU.bypass, replica_groups=RG,
                        ins=[ag_in[li].ap().opt()],
                        outs=[ag_out[li].ap().opt()])
                    return
                # ---------- layer-3 tail ----------
                for c in range(ntile):
                    pk = ep.tile([P, 4], BF16, tag="pk", name="pk")
                    nc.vector.tensor_copy(pk[:, 0:2], s12[c][:])
                    hif = ep.tile([P, 2], F32, tag="hif3", name="hif3")
                    nc.vector.tensor_copy(hif[:], pk[:, 0:2])
                    lo = ep.tile([P, 2], F32, tag="lo3", name="lo3")
                    nc.vector.tensor_tensor(out=lo[:], in0=s12[c][:],
                                            in1=hif[:], op=ALU.subtract)
                    nc.vector.tensor_copy(pk[:, 2:4], lo[:])
                    nc.sync.dma_start(st3_in.ap()[c * P:(c + 1) * P, :],
                                      pk[:])
                nc.gpsimd.collective_compute(
                    "AllGather", ALU.bypass, replica_groups=RG,
                    ins=[st3_in.ap().opt()], outs=[st3_out.ap().opt()])
                with tc.tile_pool(name="tail", bufs=2) as tp:
                    for c in range(4):
                        acc = tp.tile([P, 2], F32, tag="stacc", name="stacc")
                        nc.vector.memset(acc[:], 0.0)
                        for k in range(NCORES):
                            st = tp.tile([P, 4], BF16, tag="st", name="st")
                            r0 = (k * 4 + c) * P
                            nc.sync.dma_start(st[:],
                                              st3_out.ap()[r0:r0 + P, :])
                            nc.vector.tensor_tensor(
                                out=acc[:], in0=acc[:], in1=st[:, 0:2],
                                op=ALU.add)
                            nc.vector.tensor_tensor(
                                out=acc[:], in0=acc[:], in1=st[:, 2:4],
                                op=ALU.add)
                        A, B = compute_AB(tp, acc, 2, c)
                        pool_c = tp.tile([P, 2], F32, tag="poolc", name="poolc")
                        nc.scalar.activation(pool_c[:], sgm[c][:], ACTF.Relu,
                                             bias=B[:], scale=A[:])
                        nc.sync.dma_start(pl_in.ap()[c * P:(c + 1) * P, :],
                                          pool_c[:])
                    nc.gpsimd.collective_compute(
                        "AllGather", ALU.bypass, replica_groups=RG,
                        ins=[pl_in.ap().opt()], outs=[pl_out.ap().opt()])
                    psfc = pp.tile([16, 10], F32, tag="pt", name="psfc")
                    for c in range(4):
                        pooled = tp.tile([P, 16], F32, tag="pooled", name="pooled")
                        for k in range(NCORES):
                            r0 = (k * 4 + c) * P
                            nc.sync.dma_start(pooled[:, 2 * k:2 * k + 2],
                                              pl_out.ap()[r0:r0 + P, :])
                        nc.tensor.matmul(out=psfc[:], lhsT=pooled[:],
                                         rhs=fcw_t[c][:], start=(c == 0),
                                         stop=False, skip_group_check=True)
                    nc.tensor.matmul(out=psfc[:], lhsT=ones16[:],
                                     rhs=fcb_t[:], start=False, stop=True,
                                     skip_group_check=True)
                    ot = tp.tile([16, 10], F32, tag="ot", name="ot")
                    nc.vector.tensor_copy(ot[:], psfc[:])
                    nc.sync.dma_start(out.ap(), ot[:])

        def bn_x_phase(l):
            li = l - 1
            ntile = NTILES[li - 1]
            x_tiles = {}
            xp = tc.tile_pool(name=f"x{l}", bufs=1)
            xpool = xp.__enter__()
            with tc.tile_pool(name=f"bnx{l}", bufs=3) as bp:
                AB = {}
                for c in range(ntile):
                    acc = bp.tile([P, 2], F32, tag="acc", name="acc")
                    nc.vector.memset(acc[:], 0.0)
                    for k in range(NCORES):
                        st = bp.tile([P, 4], BF16, tag="st", name="st")
                        r0 = (k * ntile + c) * P
                        nc.sync.dma_start(
                            st[:],
                            ag_out[li - 1].ap()[r0:r0 + P, NLOC:NLOC + 4])
                        nc.vector.tensor_tensor(out=acc[:], in0=acc[:],
                                                in1=st[:, 0:2], op=ALU.add)
                        nc.vector.tensor_tensor(out=acc[:], in0=acc[:],
                                                in1=st[:, 2:4], op=ALU.add)
                    AB[c] = compute_AB(bp, acc, li - 1, c)
                for k in range(NCORES):
                    for c in range(ntile):
                        blk = bp.tile([P, NLOC], BF16, tag="blk", name="blk")
                        r0 = (k * ntile + c) * P
                        nc.sync.dma_start(
                            blk[:], ag_out[li - 1].ap()[r0:r0 + P, 0:NLOC])
                        xt = xpool.tile([P, NLOC], BF16, tag=f"x{k}_{c}", name=f"x{k}_{c}")
                        A, B = AB[c]
                        nc.scalar.activation(xt[:], blk[:], ACTF.Relu,
                                             bias=B[:], scale=A[:])
                        x_tiles[(k, c)] = xt
            return x_tiles, xp

        for _rep in range(repeat):
            if stages >= 1:
                dense_phase(1, None)
            if stages >= 2:
                edge_phase(1)
            if stages >= 3:
                x2, xp2 = bn_x_phase(2)
                dense_phase(2, x2)
                xp2.__exit__(None, None, None)
            if stages >= 4:
                edge_phase(2)
            if stages >= 5:
                x3, xp3 = bn_x_phase(3)
                dense_phase(3, x3)
                xp3.__exit__(None, None, None)
            if stages >= 6:
                edge_phase(3)

    nc.compile()
    return nc


# ================= host preprocessing =================

def prepare(inputs):
    x = np.asarray(inputs["x"], np.float32)
    ei = np.asarray(inputs["edge_index"])
    batch = np.asarray(inputs["batch"]).astype(np.int64)
    N = x.shape[0]
    assert np.all(np.diff(batch) >= 0), "batch must be sorted"
    seg_sizes = np.bincount(batch, minlength=16)
    SEGLEN = int(np.ceil(max(seg_sizes.max(), 1) / P) * P)
    NLOC = 2 * SEGLEN
    NW = NLOC // P
    NPD = 16 * SEGLEN
    assert NPD < 32768, "device node ids must fit int16 for dma_gather"
    seg_start = np.zeros(16, np.int64)
    seg_start[1:] = np.cumsum(seg_sizes)[:-1]
    dev_of = batch * SEGLEN + (np.arange(N) - seg_start[batch])

    src = np.concatenate([ei[0].astype(np.int64), np.arange(N)])
    dst = np.concatenate([ei[1].astype(np.int64), np.arange(N)])
    sdev = dev_of[src]
    ddev = dev_of[dst]
    core = ddev // NLOC
    dloc = ddev % NLOC
    win = dloc // P
    wloc = dloc % P
    key = core * NW + win
    counts = np.bincount(key, minlength=NCORES * NW)
    cw = counts.reshape(NCORES, NW)
    nsubs = np.maximum(1, np.ceil(cw.max(axis=0) / P).astype(np.int64))
    csum = np.zeros(NW + 1, np.int64)
    csum[1:] = np.cumsum(nsubs)
    TS = int(csum[-1])

    perm = np.argsort(key, kind="stable")
    gstart = np.zeros(NCORES * NW, np.int64)
    gstart[1:] = np.cumsum(counts)[:-1]
    pos = np.arange(len(perm)) - gstart[key[perm]]
    kperm = key[perm]
    wbase = (csum[:-1] * P)[kperm % NW]
    slot = (kperm // NW) * (TS * P) + wbase + pos

    src_slot = np.zeros(NCORES * TS * P, np.int16)
    dl_slot = np.full(NCORES * TS * P, 300.0, np.float32)
    src_slot[slot] = sdev[perm].astype(np.int16)
    dl_slot[slot] = wloc[perm].astype(np.float32)
    src_slot = src_slot.reshape(NCORES, TS * P)
    dl_slot = dl_slot.reshape(NCORES, TS * P)

    wcs, bngs, bnbs = [], [], []
    for l, (cin, C) in enumerate([(3, 16), (128, 32), (256, 64)], start=1):
        W = np.asarray(inputs[f"W{l}"], np.float32)
        a_s = np.asarray(inputs[f"as{l}"], np.float32)
        a_d = np.asarray(inputs[f"ad{l}"], np.float32)
        HC = H * C
        Asm = np.zeros((HC, H), np.float32)
        Adm = np.zeros((HC, H), np.float32)
        for hd in range(H):
            Asm[hd * C:(hd + 1) * C, hd] = a_s[hd]
            Adm[hd * C:(hd + 1) * C, hd] = a_d[hd]
        wcat = np.concatenate([W, W @ Asm, W @ Adm], axis=1)  # [cin, HC+16]
        nk = NKS[l - 1]
        if nk > 1:
            wcat = np.concatenate(
                [wcat[c * P:(c + 1) * P] for c in range(nk)], axis=1)
        wcs.append(np.ascontiguousarray(wcat).astype(ml_dtypes.bfloat16))
        nt = HC // P
        bngs.append(np.ascontiguousarray(
            np.asarray(inputs[f"g{l}"], np.float32).reshape(nt, P).T))
        bnbs.append(np.ascontiguousarray(
            np.asarray(inputs[f"be{l}"], np.float32).reshape(nt, P).T))
    fcwb = np.concatenate(
        [np.asarray(inputs["fcW"], np.float32),
         np.asarray(inputs["fcb"], np.float32)[None, :]], axis=0)

    x_dev = np.zeros((NPD, 3), np.float32)
    x_dev[dev_of] = x
    xT = np.ascontiguousarray(x_dev.T).astype(ml_dtypes.bfloat16)

    ioro = np.broadcast_to(np.arange(P, dtype=np.float32)[None, :], (P, P))
    ioro = np.ascontiguousarray(ioro).astype(ml_dtypes.bfloat16)
    ioco = np.arange(P, dtype=np.float32)[:, None].astype(ml_dtypes.bfloat16)
    idnt = np.eye(P, dtype=np.float32)

    in_maps = []
    for k in range(NCORES):
        sf = src_slot[k]
        idx_tile = np.zeros((P, TS * 8), np.int16)
        w16 = sf.reshape(TS * 8, 16).T
        idx_tile[:16] = w16
        idx_tile[16:] = np.tile(w16, (7, 1))
        dl = dl_slot[k]
        dloc_cols = np.ascontiguousarray(
            dl.reshape(TS, P).T).astype(ml_dtypes.bfloat16)
        drep = np.broadcast_to(dl.reshape(1, TS * P), (P, TS * P))
        drep = np.ascontiguousarray(drep).astype(ml_dtypes.bfloat16)
        dids = (k * NLOC + np.arange(NW)[None, :] * P
                + np.arange(P)[:, None]).astype(np.int32)
        pm = np.zeros((1, NLOC), np.float32)
        for s in (2 * k, 2 * k + 1):
            off = (s - 2 * k) * SEGLEN
            pm[0, off + seg_sizes[s]: off + SEGLEN] = -1e30
        im = {
            "xT": xT, "fcwb": fcwb.astype(np.float32),
            "srcs": idx_tile, "dstloc": dloc_cols, "dstrep": drep,
            "dstids": np.ascontiguousarray(dids), "pmask": pm,
            "ioro": ioro, "ioco": np.ascontiguousarray(ioco), "idnt": idnt,
        }
        for l in (1, 2, 3):
            im[f"wc{l}"] = wcs[l - 1]
            im[f"bng{l}"] = bngs[l - 1]
            im[f"bnb{l}"] = bnbs[l - 1]
        in_maps.append(im)
    return SEGLEN, tuple(int(v) for v in nsubs), N, in_maps


_CACHE = {}


def _get_nc(SEGLEN, NSUB, n_real):
    key = (SEGLEN, NSUB, n_real)
    if key not in _CACHE:
        nc = build_gnn(SEGLEN, NSUB, n_real)
        hoist_excess_waits(nc)
        _CACHE[key] = nc
    return _CACHE[key]


def kernel(**inputs):
    patch_walrus()
    SEGLEN, NSUB, n_real, in_maps = prepare(inputs)
    nc = _get_nc(SEGLEN, NSUB, n_real)
    from concourse import bass_utils
    res = bass_utils.run_bass_kernel_spmd(
        nc, in_maps, core_ids=list(range(NCORES)))
    return np.asarray(res.results[0]["out"]).astype(np.float32)



# revision 26
# speedup vs baseline: 1.6393x; 1.6393x over previous
"""Trainium2 Bass kernel for 3-layer GAT + BN/ReLU + global max pool + FC.

v2. Sharding: nodes renumbered so each of the 16 graphs (batch segments) gets
a fixed padded block of SEGLEN nodes; core k owns segments {2k,2k+1} (NLOC
contiguous device nodes). Edges partitioned by destination core, grouped in
128-dst windows; segmented softmax-sum + feature aggregation are PE matmuls
against host-precomputed 0/1 selector matrices (selc input, SBUF-resident).
Features are stored c-major (col = c*H+h) so the per-edge alpha multiply hits
the DVE 2x perf mode. Per-edge source rows come from one dma_gather per
window out of a per-layer DRAM table [h bf16 | als]; per-edge dst alpha from
one batched 8-elem dma_gather per layer out of aldtab. Params replicated; two
chunked AllGathers per layer carry channel-major pre-BN output (chunk B also
carries hi/lo-packed BN partial sums) so network overlaps edge compute.
"""
import sys

sys.path.insert(0, "/opt/trn_rl_repo")
sys.path.insert(0, "/opt/trn_rl_repo/concourse")

import numpy as np
import ml_dtypes

import concourse.bass as bass
import concourse.tile as tile
import concourse.mybir as mybir
from concourse import bacc
from concourse.bass import IndirectOffsetOnAxis

P = 128
NCORES = 8
H = 8
EPS_BN = 1e-5
NEG_SLOPE = 0.2
F32 = mybir.dt.float32
BF16 = mybir.dt.bfloat16
FP8 = mybir.dt.float8e4

CINS = [3, 128, 256]
CS = [16, 32, 64]
HCS = [128, 256, 512]
ROWES = [256, 384, 640]   # table row stride in bf16 elems (256B units)
WRITEWS = [144, 272, 528]  # written row prefix: h (c-major) + als + ald
NTILES = [1, 2, 4]        # HC/128 per layer
NKS = [1, 1, 2]           # K-tiles per dense matmul
GRP = 4                   # table-write batching (tiles per DMA)

AX = mybir.AxisListType
ALU = mybir.AluOpType
ACTF = mybir.ActivationFunctionType


def apx(sl, dims):
    """AP with custom free dims [(step, count), ...] on a tile slice's base."""
    return bass.AP(sl.tensor, sl.offset,
                   [list(sl.ap[0])] + [list(d) for d in dims])


def apd(t, off, dims):
    """Raw AP on a DRAM tensor with explicit dims."""
    return bass.AP(t, off, [list(d) for d in dims])


def hoist_excess_waits(nc, cap=1):
    cnt = [0]
    total = 0
    for f in nc.m.functions:
        for b in f.blocks:
            if not any(
                i.sync_info is not None and i.sync_info.on_wait
                and len(i.sync_info.on_wait) > cap
                for i in b.instructions
            ):
                continue
            new_insts = []
            for inst in b.instructions:
                si = inst.sync_info
                if si is not None and si.on_wait and len(si.on_wait) > cap:
                    waits = list(si.on_wait)
                    for w in waits[:-cap]:
                        cnt[0] += 1
                        new_insts.append(
                            mybir.InstNoOp(
                                name=f"hw-{cnt[0]}",
                                engine=inst.engine,
                                ins=[],
                                outs=[],
                                sync_info=mybir.SyncInfo(on_wait=[w],
                                                         on_update=[]),
                            )
                        )
                        total += 1
                    inst.sync_info = mybir.SyncInfo(
                        on_wait=waits[-cap:], on_update=list(si.on_update))
                new_insts.append(inst)
            b.instructions = new_insts
    return total


def patch_walrus():
    from concourse import bass_utils as bu
    if getattr(bu, "_dge_patched", False):
        return
    orig = bu.get_walrus_args

    def patched(*a, **k):
        return orig(*a, **k) + [
            "--dge-levels=io,spill_reload,scalar_dynamic_offset,"
            "vector_dynamic_offsets,dst_reduce"
        ]

    bu.get_walrus_args = patched
    bu._dge_patched = True


def build_gnn(SEGLEN, NSUB, n_real, stages=99, repeat=1, dbg=False):
    nsubs = list(NSUB) if isinstance(NSUB, (list, tuple)) else None
    NLOC = 2 * SEGLEN
    NW = NLOC // P
    NPD = 16 * SEGLEN
    SEGW = NW // 2
    if nsubs is None:
        nsubs = [NSUB] * NW
    csum = [0]
    for v in nsubs:
        csum.append(csum[-1] + v)
    TS = csum[-1]  # total subchunks per core
    NSMAX = max(nsubs)
    CHW = NW // 2            # windows in AllGather chunk A
    CAC = CHW * P            # chunk A columns
    CBC = NLOC - CAC + 4     # chunk B columns (incl packed stats)

    nc = bacc.Bacc("TRN2", target_bir_lowering=False, debug=False,
                   num_devices=NCORES, num_swdge_queues=2)

    # ---------------- I/O ----------------
    xT = nc.dram_tensor("xT", [3, NPD], BF16, kind="ExternalInput")
    wc = [nc.dram_tensor(f"wc{l}", [P if l > 1 else 3,
                                    NKS[l - 1] * (HCS[l - 1] + 16)], BF16,
                         kind="ExternalInput") for l in (1, 2, 3)]
    bng = [nc.dram_tensor(f"bng{l}", [P, NTILES[l - 1]], F32,
                          kind="ExternalInput") for l in (1, 2, 3)]
    bnb = [nc.dram_tensor(f"bnb{l}", [P, NTILES[l - 1]], F32,
                          kind="ExternalInput") for l in (1, 2, 3)]
    fcwb = nc.dram_tensor("fcwb", [513, 10], F32, kind="ExternalInput")
    srcs = nc.dram_tensor("srcs", [P, TS * 8], mybir.dt.int16,
                          kind="ExternalInput")
    dstids = nc.dram_tensor("dstids", [P, NW], mybir.dt.int32,
                            kind="ExternalInput")
    selc = nc.dram_tensor("selc", [P, TS * P], FP8, kind="ExternalInput")
    selt = nc.dram_tensor("selt", [P, TS * P], FP8, kind="ExternalInput")
    pmask = nc.dram_tensor("pmask", [1, NLOC], F32, kind="ExternalInput")
    pmrep = nc.dram_tensor("pmrep", [P, NLOC], BF16, kind="ExternalInput")
    idnt = nc.dram_tensor("idnt", [P, P], F32, kind="ExternalInput")
    out = nc.dram_tensor("out", [16, 10], F32, kind="ExternalOutput")

    # ---------------- internal DRAM ----------------
    tables = [nc.dram_tensor(f"table{l}", [NPD, ROWES[l - 1]], BF16)
              for l in (1, 2, 3)]
    agA_in = [nc.dram_tensor(f"agA{l}_in", [NTILES[l - 1] * P, CAC], BF16)
              for l in (1, 2)]
    agA_out = [nc.dram_tensor(f"agA{l}_out",
                              [NCORES * NTILES[l - 1] * P, CAC], BF16,
                              addr_space="Shared")
               for l in (1, 2)]
    agB_in = [nc.dram_tensor(f"agB{l}_in", [NTILES[l - 1] * P, CBC], BF16)
              for l in (1, 2)]
    agB_out = [nc.dram_tensor(f"agB{l}_out",
                              [NCORES * NTILES[l - 1] * P, CBC], BF16,
                              addr_space="Shared")
               for l in (1, 2)]
    if dbg:
        dbgtab = nc.dram_tensor("dbgtab", [NPD, ROWES[0]], BF16,
                                kind="ExternalOutput")
        dbgz = nc.dram_tensor("dbgz", [P, CAC + CBC - 4], BF16,
                              kind="ExternalOutput")
    st3_in = nc.dram_tensor("st3_in", [4 * P, 4], F32)
    st3_out = nc.dram_tensor("st3_out", [NCORES * 4 * P, 4], F32,
                             addr_space="Shared")

    RG = [list(range(NCORES))]

    with tile.TileContext(nc) as tc, tc.tile_pool(name="const", bufs=1) as cp:
        # ---------- constants ----------
        ident = cp.tile([P, P], F32, tag="idnt", name="idnt")
        nc.sync.dma_start(ident[:], idnt.ap())
        onesf = cp.tile([1, P], F32, tag="onesf", name="onesf")
        nc.vector.memset(onesf[:], 1.0)
        ones16 = cp.tile([1, 16], F32, tag="ones16", name="ones16")
        nc.vector.memset(ones16[:], 1.0)

        wc_t = []
        for i in range(3):
            t = cp.tile([P if i > 0 else 3, NKS[i] * (HCS[i] + 16)], BF16,
                        tag=f"wc{i}", name=f"wc{i}")
            nc.sync.dma_start(t[:], wc[i].ap())
            wc_t.append(t)
        bng_t, bnb_t = [], []
        for i in range(3):
            tg = cp.tile([P, NTILES[i]], F32, tag=f"bng{i}", name=f"bng{i}")
            tb = cp.tile([P, NTILES[i]], F32, tag=f"bnb{i}", name=f"bnb{i}")
            nc.sync.dma_start(tg[:], bng[i].ap())
            nc.sync.dma_start(tb[:], bnb[i].ap())
            bng_t.append(tg)
            bnb_t.append(tb)
        fcw_t = []
        for c in range(4):
            t = cp.tile([P, 10], F32, tag=f"fcw{c}", name=f"fcw{c}")
            nc.sync.dma_start(t[:], fcwb.ap()[c * P:(c + 1) * P, :])
            fcw_t.append(t)
        fcb_t = cp.tile([1, 10], F32, tag="fcb", name="fcb")
        nc.sync.dma_start(fcb_t[:], fcwb.ap()[512:513, :])
        pmask_t = cp.tile([1, NLOC], F32, tag="pmask", name="pmask")
        nc.sync.dma_start(pmask_t[:], pmask.ap())
        pmrep_t = cp.tile([P, NLOC], BF16, tag="pmrep", name="pmrep")
        nc.sync.dma_start(pmrep_t[:], pmrep.ap())
        idx_t = cp.tile([P, TS * 8], mybir.dt.int16, tag="idx", name="idx")
        nc.sync.dma_start(idx_t[:], srcs.ap())
        dids_t = cp.tile([P, NW], mybir.dt.int32, tag="dids", name="dids")
        nc.sync.dma_start(dids_t[:], dstids.ap())

        def compute_AB(pool, s12, gi, c):
            mu = pool.tile([P, 1], F32, tag="mu", name="mu")
            nc.vector.tensor_scalar(mu[:], s12[:, 0:1], 1.0 / n_real, None,
                                    op0=ALU.mult)
            ex2 = pool.tile([P, 1], F32, tag="ex2", name="ex2")
            nc.vector.tensor_scalar(ex2[:], s12[:, 1:2], 1.0 / n_real, None,
                                    op0=ALU.mult)
            var = pool.tile([P, 1], F32, tag="var", name="var")
            nc.vector.tensor_tensor(out=var[:], in0=mu[:], in1=mu[:],
                                    op=ALU.mult)
            nc.vector.tensor_tensor(out=var[:], in0=ex2[:], in1=var[:],
                                    op=ALU.subtract)
            nc.vector.tensor_scalar(var[:], var[:], EPS_BN, None, op0=ALU.add)
            sd = pool.tile([P, 1], F32, tag="sd", name="sd")
            nc.scalar.sqrt(sd[:], var[:])
            rs = pool.tile([P, 1], F32, tag="rs", name="rs")
            nc.vector.reciprocal(rs[:], sd[:])
            A = pool.tile([P, 1], F32, tag="A", name="A")
            nc.vector.tensor_tensor(out=A[:], in0=rs[:],
                                    in1=bng_t[gi][:, c:c + 1], op=ALU.mult)
            B = pool.tile([P, 1], F32, tag="B", name="B")
            nc.vector.tensor_tensor(out=B[:], in0=mu[:], in1=A[:],
                                    op=ALU.mult)
            nc.vector.tensor_tensor(out=B[:], in0=bnb_t[gi][:, c:c + 1],
                                    in1=B[:], op=ALU.subtract)
            return A, B

        def dense_phase(l):
            """h = x @ W for all NPD rows (replicated on every core).

            Writes [h(c-major) | als] prefix of each table row, plus aldtab.
            For l>=2, x is read per-core-block from the layer-(l-1) AllGather
            chunks with BN+ReLU applied on the fly.
            """
            li = l - 1
            HC, ROWE, WW = HCS[li], ROWES[li], WRITEWS[li]
            nk = NKS[li]
            WCW = HC + 16
            fused = WCW <= 512
            with (
                tc.tile_pool(name=f"d{l}", bufs=3) as dp,
                tc.tile_pool(name=f"dx{l}", bufs=2) as xp,
                tc.tile_pool(name=f"dps{l}", bufs=3, space="PSUM") as pp,
            ):
                AB = None
                if l > 1:
                    pli = li - 1
                    pntile = NTILES[pli]
                    pCBC = CBC
                    AB = []
                    for c in range(pntile):
                        stt = dp.tile([P, 8, 4], BF16, tag="stt", name="stt")
                        nc.sync.dma_start(
                            stt[:],
                            apd(agB_out[pli], c * P * pCBC + (pCBC - 4),
                                [[pCBC, P], [pntile * P * pCBC, 8], [1, 4]]))
                        acc = dp.tile([P, 2], F32, tag="acc", name="acc")
                        nc.vector.memset(acc[:], 0.0)
                        for k in range(NCORES):
                            nc.vector.tensor_tensor(
                                out=acc[:], in0=acc[:], in1=stt[:, k, 0:2],
                                op=ALU.add)
                            nc.vector.tensor_tensor(
                                out=acc[:], in0=acc[:], in1=stt[:, k, 2:4],
                                op=ALU.add)
                        AB.append(compute_AB(dp, acc, li - 1, c))

                for k in range(NCORES):
                    if l == 1:
                        xts = None
                    else:
                        pli = li - 1
                        pntile = NTILES[pli]
                        xts = []
                        for c in range(nk):
                            blk = xp.tile([P, NLOC], BF16, tag=f"blk{c}",
                                          name=f"blk{c}")
                            r0 = (k * pntile + c) * P
                            nc.sync.dma_start(
                                blk[:, 0:CAC],
                                agA_out[pli].ap()[r0:r0 + P, :])
                            nc.sync.dma_start(
                                blk[:, CAC:NLOC],
                                agB_out[pli].ap()[r0:r0 + P, 0:NLOC - CAC])
                            xt = xp.tile([P, NLOC], BF16, tag=f"x{c}",
                                         name=f"x{c}")
                            A, B = AB[c]
                            nc.scalar.activation(xt[:], blk[:], ACTF.Relu,
                                                 bias=B[:], scale=A[:])
                            xts.append(xt)
                    stage = None
                    for jj in range(NW):
                        gi = jj % GRP
                        if gi == 0:
                            ng = min(GRP, NW - jj)
                            stage = dp.tile([P, GRP, ROWE], BF16, tag="stage",
                                            name="stage")
                        psA = pp.tile([P, WCW if fused else HC], F32,
                                      tag="psA", name="psA")
                        psB = None if fused else pp.tile([P, 16], F32,
                                                         tag="psB", name="psB")
                        for c in range(nk):
                            if l == 1:
                                lhs = xT_t[:, k * NLOC + jj * P:
                                           k * NLOC + jj * P + P]
                            else:
                                lhs = xts[c][:, jj * P:(jj + 1) * P]
                            if fused:
                                nc.tensor.matmul(
                                    out=psA[:], lhsT=lhs,
                                    rhs=wc_t[li][:, c * WCW:(c + 1) * WCW],
                                    start=(c == 0), stop=(c == nk - 1))
                            else:
                                rhsW = wc_t[li][:, c * WCW:c * WCW + HC]
                                rhsb = wc_t[li][:, c * WCW + HC:(c + 1) * WCW]
                                nc.tensor.matmul(out=psA[:], lhsT=lhs,
                                                 rhs=rhsW, start=(c == 0),
                                                 stop=(c == nk - 1))
                                nc.tensor.matmul(out=psB[:], lhsT=lhs,
                                                 rhs=rhsb, start=(c == 0),
                                                 stop=(c == nk - 1))
                        def cpy(dst, src, _a=(jj % 2 == 1)):
                            if _a:
                                nc.scalar.copy(dst, src)
                            else:
                                nc.vector.tensor_copy(dst, src)
                        if fused:
                            cpy(stage[:, gi, 0:WW], psA[:, 0:WW])
                        else:
                            cpy(stage[:, gi, 0:HC], psA[:])
                            cpy(stage[:, gi, HC:HC + 16], psB[:])
                        if gi == ng - 1:
                            base = k * NLOC + (jj - gi) * P
                            nc.sync.dma_start(
                                bass.AP(tables[li], base * ROWE,
                                        [[ROWE, P], [P * ROWE, ng], [1, WW]]),
                                apx(stage[:], [[ROWE, ng], [1, WW]]))

        def edge_phase(l, selc_t, selt_t):
            """Software-pipelined window loop (depth PD).

            Stage A(w): gather g, psew matmuls, ew chain -> wbf.
            Stage B(w): wha, psf/psd matmuls, softmax div, transpose, zT/stats.
            For l==3 the selector tiles are streamed from DRAM per window.
            """
            li = l - 1
            HC, C, ROWE = HCS[li], CS[li], ROWES[li]
            ntile = NTILES[li]
            PD = 2
            stream_sel = selc_t is None
            with (
                tc.tile_pool(name=f"e{l}", bufs=PD + 2) as ep,
                tc.tile_pool(name=f"ew{l}", bufs=2) as wp,
                tc.tile_pool(name=f"eg{l}", bufs=PD + 1) as gp,
                tc.tile_pool(name=f"es{l}", bufs=PD + 1) as slp,
                tc.tile_pool(name=f"ez{l}", bufs=1) as zp,
                tc.tile_pool(name=f"eps{l}", bufs=2, space="PSUM") as pp,
                tc.tile_pool(name=f"epw{l}", bufs=2, space="PSUM") as ppw,
                tc.tile_pool(name=f"epf{l}", bufs=2, space="PSUM") as ppf,
            ):
                if dbg and l == 1:
                    nc.sync.dma_start(dbgtab.ap(), tables[0].ap())
                aldbf_all = zp.tile([P, NW, 8], BF16, tag="aldbf",
                                    name="aldbf")
                for w in range(NW):
                    nc.gpsimd.indirect_dma_start(
                        out=aldbf_all[:, w, :],
                        out_offset=None,
                        in_=tables[li].ap(),
                        in_offset=IndirectOffsetOnAxis(
                            ap=dids_t[:, w:w + 1], axis=0),
                        element_offset=HC + 8,
                    )
                zTA = [zp.tile([P, CAC], BF16, tag=f"zTA{c}",
                               name=f"zTA{c}") for c in range(ntile)]
                zTB = [zp.tile([P, CBC if l < 3 else NLOC - CAC], BF16,
                               tag=f"zTB{c}", name=f"zTB{c}")
                       for c in range(ntile)]

                live = {}

                def stage_a(w):
                    ns = nsubs[w]
                    base = csum[w]
                    g = gp.tile([P, NSMAX, ROWE], BF16, tag="g",
                                name="g")[:, 0:ns, :]
                    nc.gpsimd.dma_gather(
                        out_ap=g[:],
                        in_ap=tables[li].ap(),
                        idxs_ap=idx_t[:, base * 8:(base + ns) * 8],
                        num_idxs=ns * P,
                        num_idxs_reg=ns * P,
                        elem_size=ROWE,
                        single_packet=False,
                        queue_num=w % 2,
                    )
                    if stream_sel:
                        scw = slp.tile([P, NSMAX * P], FP8, tag="scw",
                                       name="scw")[:, 0:ns * P]
                        nc.sync.dma_start(
                            scw[:], selc.ap()[:, base * P:(base + ns) * P])
                        stw = slp.tile([P, NSMAX * P], FP8, tag="stw",
                                       name="stw")[:, 0:ns * P]
                        nc.sync.dma_start(
                            stw[:], selt.ap()[:, base * P:(base + ns) * P])
                    else:
                        scw = selc_t[:, base * P:(base + ns) * P]
                        stw = selt_t[:, base * P:(base + ns) * P]
                    psew = ppw.tile([P, NSMAX * 8], F32, tag="psew",
                                    name="psew")[:, 0:ns * 8]
                    for s in range(ns):
                        nc.tensor.matmul(
                            out=psew[:, s * 8:(s + 1) * 8],
                            lhsT=stw[:, s * P:(s + 1) * P],
                            rhs=aldbf_all[:, w, :], start=True, stop=True)
                    ew = ep.tile([P, NSMAX, 8], BF16, tag="ew",
                                 name="ew")[:, 0:ns, :]
                    nc.vector.tensor_tensor(
                        out=ew[:],
                        in0=apx(g[:, 0, HC:HC + 8], [(ROWE, ns), (1, 8)]),
                        in1=apx(psew[:], [(8, ns), (1, 8)]),
                        op=ALU.add)
                    ew2 = ep.tile([P, NSMAX, 8], BF16, tag="ew2",
                                  name="ew2")[:, 0:ns, :]
                    nc.vector.tensor_scalar(ew2[:], ew[:], NEG_SLOPE, None,
                                            op0=ALU.mult)
                    nc.vector.tensor_tensor(out=ew2[:], in0=ew[:], in1=ew2[:],
                                            op=ALU.max)
                    wbf = ep.tile([P, NSMAX, 8], BF16, tag="wbf",
                                  name="wbf")[:, 0:ns, :]
                    nc.scalar.activation(wbf[:], ew2[:], ACTF.Exp)
                    live[w] = (g, scw, wbf)

                def stage_b(w):
                    ns = nsubs[w]
                    base = csum[w]
                    g, scw, wbf = live.pop(w)
                    merged = HC + 8 <= 512
                    psf = ppf.tile([P, HC + 8 if merged else HC], F32,
                                   tag="psf", name="psf")
                    psd = None if merged else pp.tile([P, 8], F32, tag="psd",
                                                      name="psd")
                    WHW = HC + 8 if merged else HC
                    wha = wp.tile([P, NSMAX, WHW], BF16, tag="wha",
                                  name="wha")[:, 0:ns, :]
                    nc.vector.tensor_tensor(
                        out=apx(wha[:], [(WHW, ns), (H, C), (1, H)]),
                        in0=apx(g[:, 0, 0:HC], [(ROWE, ns), (H, C), (1, H)]),
                        in1=apx(wbf[:, 0, :], [(8, ns), (0, C), (1, 8)]),
                        op=ALU.mult)
                    if merged:
                        nc.vector.tensor_copy(
                            apx(wha[:, 0, HC:HC + 8], [(WHW, ns), (1, 8)]),
                            apx(wbf[:, 0, :], [(8, ns), (1, 8)]))
                    for s in range(ns):
                        sl = scw[:, s * P:(s + 1) * P]
                        nc.tensor.matmul(
                            out=psf[:], lhsT=sl, rhs=wha[:, s, :],
                            start=(s == 0), stop=(s == ns - 1))
                        if not merged:
                            nc.tensor.matmul(
                                out=psd[:], lhsT=sl, rhs=wbf[:, s, :],
                                start=(s == 0), stop=(s == ns - 1))

                    den = ep.tile([P, 8], F32, tag="den", name="den")
                    nc.vector.tensor_scalar(
                        den[:], psf[:, HC:HC + 8] if merged else psd[:],
                        1e-16, None, op0=ALU.add)
                    rec = ep.tile([P, 8], F32, tag="rec", name="rec")
                    nc.vector.reciprocal(rec[:], den[:])
                    z = ep.tile([P, HC], F32, tag="z", name="z")
                    nc.vector.tensor_tensor(
                        out=apx(z[:], [(H, C), (1, H)]),
                        in0=apx(psf[:], [(H, C), (1, H)]),
                        in1=apx(rec[:], [(0, C), (1, H)]),
                        op=ALU.mult)

                    for c in range(ntile):
                        pt = pp.tile([P, P], F32, tag="pt", name="pt")
                        nc.tensor.transpose(pt[:], z[:, c * P:(c + 1) * P],
                                            ident[:])
                        if w < CHW:
                            nc.scalar.copy(
                                zTA[c][:, w * P:(w + 1) * P], pt[:])
                        else:
                            nc.scalar.copy(
                                zTB[c][:, (w - CHW) * P:
                                       (w - CHW + 1) * P], pt[:])

                    if l < 3 and w == CHW - 1:
                        for c in range(ntile):
                            nc.sync.dma_start(
                                agA_in[li].ap()[c * P:(c + 1) * P, :],
                                zTA[c][:])
                        nc.gpsimd.collective_compute(
                            "AllGather", ALU.bypass, replica_groups=RG,
                            ins=[agA_in[li].ap().opt()],
                            outs=[agA_out[li].ap().opt()])

                for w in range(NW + PD):
                    if w < NW:
                        stage_a(w)
                    if w >= PD:
                        stage_b(w - PD)

                if l < 3:
                    for c in range(ntile):
                        s1 = ep.tile([P, 1], F32, tag="s1", name="s1")
                        s1b = ep.tile([P, 1], F32, tag="s1b", name="s1b")
                        nc.vector.reduce_sum(s1[:], zTA[c][:], axis=AX.X)
                        nc.vector.reduce_sum(s1b[:], zTB[c][:, 0:NLOC - CAC],
                                             axis=AX.X)
                        scr = ep.tile([P, CAC], BF16, tag="scr", name="scr")
                        s2 = ep.tile([P, 1], F32, tag="s2", name="s2")
                        s2b = ep.tile([P, 1], F32, tag="s2b", name="s2b")
                        nc.scalar.activation(scr[:], zTA[c][:], ACTF.Square,
                                             accum_out=s2[:])
                        nc.scalar.activation(scr[:, 0:NLOC - CAC],
                                             zTB[c][:, 0:NLOC - CAC],
                                             ACTF.Square, accum_out=s2b[:])
                        s12t = ep.tile([P, 2], F32, tag="s12t", name="s12t")
                        nc.vector.tensor_tensor(out=s12t[:, 0:1], in0=s1[:],
                                                in1=s1b[:], op=ALU.add)
                        nc.vector.tensor_tensor(out=s12t[:, 1:2], in0=s2[:],
                                                in1=s2b[:], op=ALU.add)
                        # pack fp32 sums as bf16 hi/lo pairs
                        nc.vector.tensor_copy(zTB[c][:, NLOC - CAC:
                                                     NLOC - CAC + 2],
                                              s12t[:])
                        hif = ep.tile([P, 2], F32, tag="hif", name="hif")
                        nc.vector.tensor_copy(hif[:],
                                              zTB[c][:, NLOC - CAC:
                                                     NLOC - CAC + 2])
                        lo = ep.tile([P, 2], F32, tag="lo", name="lo")
                        nc.vector.tensor_tensor(out=lo[:], in0=s12t[:],
                                                in1=hif[:], op=ALU.subtract)
                        nc.vector.tensor_copy(zTB[c][:, NLOC - CAC + 2:
                                                     NLOC - CAC + 4], lo[:])
                        nc.sync.dma_start(
                            agB_in[li].ap()[c * P:(c + 1) * P, :], zTB[c][:])
                    nc.gpsimd.collective_compute(
                        "AllGather", ALU.bypass, replica_groups=RG,
                        ins=[agB_in[li].ap().opt()],
                        outs=[agB_out[li].ap().opt()])
                    return
                # ---------- layer-3 strip stats + tail ----------
                s12 = []
                sgm = []
                for c in range(ntile):
                    s1 = ep.tile([P, 1], F32, tag="s1", name="s1")
                    s1b = ep.tile([P, 1], F32, tag="s1b", name="s1b")
                    nc.vector.reduce_sum(s1[:], zTA[c][:], axis=AX.X)
                    nc.vector.reduce_sum(s1b[:], zTB[c][:], axis=AX.X)
                    scr = ep.tile([P, CAC], BF16, tag="scr", name="scr")
                    s2 = ep.tile([P, 1], F32, tag="s2", name="s2")
                    s2b = ep.tile([P, 1], F32, tag="s2b", name="s2b")
                    nc.scalar.activation(scr[:], zTA[c][:], ACTF.Square,
                                         accum_out=s2[:])
                    nc.scalar.activation(scr[:, 0:NLOC - CAC], zTB[c][:],
                                         ACTF.Square, accum_out=s2b[:])
                    s12c = ep.tile([P, 2], F32, tag="s12c", name="s12c")
                    nc.vector.tensor_tensor(out=s12c[:, 0:1], in0=s1[:],
                                            in1=s1b[:], op=ALU.add)
                    nc.vector.tensor_tensor(out=s12c[:, 1:2], in0=s2[:],
                                            in1=s2b[:], op=ALU.add)
                    s12.append(s12c)
                    mz = ep.tile([P, CAC], BF16, tag="mz", name="mz")
                    sgmc = ep.tile([P, 2], F32, tag="sgmc", name="sgmc")
                    nc.vector.tensor_tensor(
                        out=mz[:], in0=zTA[c][:],
                        in1=apx(pmrep_t[:, 0:CAC], [(1, CAC)]), op=ALU.add)
                    nc.vector.reduce_max(sgmc[:, 0:1], mz[:], axis=AX.X)
                    nc.vector.tensor_tensor(
                        out=mz[:, 0:NLOC - CAC], in0=zTB[c][:],
                        in1=apx(pmrep_t[:, CAC:NLOC], [(1, NLOC - CAC)]),
                        op=ALU.add)
                    nc.vector.reduce_max(sgmc[:, 1:2], mz[:, 0:NLOC - CAC],
                                         axis=AX.X)
                    sgm.append(sgmc)
                for c in range(ntile):
                    pk = ep.tile([P, 4], F32, tag="pk", name="pk")
                    nc.vector.tensor_copy(pk[:, 0:2], s12[c][:])
                    nc.vector.tensor_copy(pk[:, 2:4], sgm[c][:])
                    nc.sync.dma_start(st3_in.ap()[c * P:(c + 1) * P, :],
                                      pk[:])
                nc.gpsimd.collective_compute(
                    "AllGather", ALU.bypass, replica_groups=RG,
                    ins=[st3_in.ap().opt()], outs=[st3_out.ap().opt()])
                with tc.tile_pool(name="tail", bufs=2) as tp:
                    psfc = pp.tile([16, 10], F32, tag="pt", name="psfc")
                    for c in range(4):
                        stt = tp.tile([P, 8, 4], F32, tag="st3t", name="st3t")
                        nc.sync.dma_start(
                            stt[:],
                            bass.AP(st3_out, c * P * 4,
                                    [[4, P], [4 * P * 4, 8], [1, 4]]))
                        acc = tp.tile([P, 2], F32, tag="stacc", name="stacc")
                        nc.vector.memset(acc[:], 0.0)
                        for k in range(NCORES):
                            nc.vector.tensor_tensor(
                                out=acc[:], in0=acc[:], in1=stt[:, k, 0:2],
                                op=ALU.add)
                        A, B = compute_AB(tp, acc, 2, c)
                        pooled = tp.tile([P, 16], F32, tag="pooled",
                                         name="pooled")
                        for k in range(NCORES):
                            nc.scalar.activation(pooled[:, 2 * k:2 * k + 2],
                                                 stt[:, k, 2:4], ACTF.Relu,
                                                 bias=B[:], scale=A[:])
                        nc.tensor.matmul(out=psfc[:], lhsT=pooled[:],
                                         rhs=fcw_t[c][:], start=(c == 0),
                                         stop=False, skip_group_check=True)
                    nc.tensor.matmul(out=psfc[:], lhsT=ones16[:],
                                     rhs=fcb_t[:], start=False, stop=True,
                                     skip_group_check=True)
                    ot = tp.tile([16, 10], F32, tag="ot", name="ot")
                    nc.vector.tensor_copy(ot[:], psfc[:])
                    nc.sync.dma_start(out.ap(), ot[:])

        for _rep in range(repeat):
            selp = tc.tile_pool(name="selp", bufs=1)
            sp = selp.__enter__()
            selc_t = sp.tile([P, TS * P], FP8, tag="selc", name="selc")
            nc.sync.dma_start(selc_t[:], selc.ap())
            selt_t = sp.tile([P, TS * P], FP8, tag="selt", name="selt")
            nc.sync.dma_start(selt_t[:], selt.ap())
            xp1 = tc.tile_pool(name="x1", bufs=1)
            xpool1 = xp1.__enter__()
            xT_t = xpool1.tile([3, NPD], BF16, tag="xT", name="xT")
            nc.sync.dma_start(xT_t[:], xT.ap())
            if stages >= 1:
                dense_phase(1)
            xp1.__exit__(None, None, None)
            if stages >= 2:
                edge_phase(1, selc_t, selt_t)
            if stages >= 3:
                dense_phase(2)
            if stages >= 4:
                edge_phase(2, selc_t, selt_t)
            selp.__exit__(None, None, None)
            if stages >= 5:
                dense_phase(3)
            if stages >= 6:
                edge_phase(3, None, None)

    nc.compile()
    return nc


# ================= host preprocessing =================

def _cmajor_idx(C):
    """idx[c*H+h] = h*C + c  (c-major column order for [C,H] heads layout)."""
    return (np.arange(H)[None, :] * C + np.arange(C)[:, None]).ravel()


def prepare(inputs):
    x = np.asarray(inputs["x"], np.float32)
    ei = np.asarray(inputs["edge_index"])
    batch = np.asarray(inputs["batch"]).astype(np.int64)
    N = x.shape[0]
    assert np.all(np.diff(batch) >= 0), "batch must be sorted"
    seg_sizes = np.bincount(batch, minlength=16)
    SEGLEN = int(np.ceil(max(seg_sizes.max(), 1) / P) * P)
    NLOC = 2 * SEGLEN
    NW = NLOC // P
    NPD = 16 * SEGLEN
    assert NPD < 32768, "device node ids must fit int16 for dma_gather"
    seg_start = np.zeros(16, np.int64)
    seg_start[1:] = np.cumsum(seg_sizes)[:-1]
    dev_of = batch * SEGLEN + (np.arange(N) - seg_start[batch])

    src = np.concatenate([ei[0].astype(np.int64), np.arange(N)])
    dst = np.concatenate([ei[1].astype(np.int64), np.arange(N)])
    sdev = dev_of[src]
    ddev = dev_of[dst]
    core = ddev // NLOC
    dloc = ddev % NLOC
    win = dloc // P
    wloc = dloc % P
    key = core * NW + win
    counts = np.bincount(key, minlength=NCORES * NW)
    cw = counts.reshape(NCORES, NW)
    nsubs = np.maximum(1, np.ceil(cw.max(axis=0) / P).astype(np.int64))
    csum = np.zeros(NW + 1, np.int64)
    csum[1:] = np.cumsum(nsubs)
    TS = int(csum[-1])

    perm = np.argsort(key, kind="stable")
    gstart = np.zeros(NCORES * NW, np.int64)
    gstart[1:] = np.cumsum(counts)[:-1]
    pos = np.arange(len(perm)) - gstart[key[perm]]
    kperm = key[perm]
    wbase = (csum[:-1] * P)[kperm % NW]
    slot = (kperm // NW) * (TS * P) + wbase + pos

    src_slot = np.zeros(NCORES * TS * P, np.int16)
    dst_slot = np.zeros(NCORES * TS * P, np.int16)
    dl_slot = np.full(NCORES * TS * P, 300, np.int64)
    src_slot[slot] = sdev[perm].astype(np.int16)
    dst_slot[slot] = ddev[perm].astype(np.int16)
    dl_slot[slot] = wloc[perm]
    src_slot = src_slot.reshape(NCORES, TS * P)
    dst_slot = dst_slot.reshape(NCORES, TS * P)
    dl_slot = dl_slot.reshape(NCORES, TS * P)

    wcs, bngs, bnbs = [], [], []
    prev_idx = None
    for l, (cin, C) in enumerate([(3, 16), (128, 32), (256, 64)], start=1):
        W = np.asarray(inputs[f"W{l}"], np.float32)
        a_s = np.asarray(inputs[f"as{l}"], np.float32)
        a_d = np.asarray(inputs[f"ad{l}"], np.float32)
        HC = H * C
        idx = _cmajor_idx(C)
        Asm = np.zeros((HC, H), np.float32)
        Adm = np.zeros((HC, H), np.float32)
        for hd in range(H):
            Asm[hd * C:(hd + 1) * C, hd] = a_s[hd]
            Adm[hd * C:(hd + 1) * C, hd] = a_d[hd]
        if prev_idx is not None:
            W = W[prev_idx, :]
        wcat = np.concatenate([W[:, idx], W @ Asm, W @ Adm], axis=1)
        nk = NKS[l - 1]
        if nk > 1:
            wcat = np.concatenate(
                [wcat[c * P:(c + 1) * P] for c in range(nk)], axis=1)
        wcs.append(np.ascontiguousarray(wcat).astype(ml_dtypes.bfloat16))
        nt = HC // P
        bngs.append(np.ascontiguousarray(
            np.asarray(inputs[f"g{l}"], np.float32)[idx].reshape(nt, P).T))
        bnbs.append(np.ascontiguousarray(
            np.asarray(inputs[f"be{l}"], np.float32)[idx].reshape(nt, P).T))
        prev_idx = idx
    fcwb = np.concatenate(
        [np.asarray(inputs["fcW"], np.float32)[prev_idx, :],
         np.asarray(inputs["fcb"], np.float32)[None, :]], axis=0)

    x_dev = np.zeros((NPD, 3), np.float32)
    x_dev[dev_of] = x
    xT = np.ascontiguousarray(x_dev.T).astype(ml_dtypes.bfloat16)

    idnt = np.eye(P, dtype=np.float32)

    in_maps = []
    for k in range(NCORES):
        def pack16(v16):
            w16 = v16.reshape(TS * 8, 16).T
            t = np.zeros((P, TS * 8), np.int16)
            t[:16] = w16
            t[16:] = np.tile(w16, (7, 1))
            return t
        idx_tile = pack16(src_slot[k])
        dl = dl_slot[k]
        sc = np.zeros((TS * P, P), np.float32)
        valid = dl < P
        sc[np.nonzero(valid)[0], dl[valid]] = 1.0
        M = sc.reshape(TS, P, P)
        scq = np.ascontiguousarray(
            M.transpose(1, 0, 2).reshape(P, TS * P)
        ).astype(ml_dtypes.float8_e4m3)
        stq = np.ascontiguousarray(
            M.transpose(2, 0, 1).reshape(P, TS * P)
        ).astype(ml_dtypes.float8_e4m3)
        dids = (k * NLOC + np.arange(NW)[None, :] * P
                + np.arange(P)[:, None]).astype(np.int32)
        pm = np.zeros((1, NLOC), np.float32)
        for s in (2 * k, 2 * k + 1):
            off = (s - 2 * k) * SEGLEN
            pm[0, off + seg_sizes[s]: off + SEGLEN] = -1e30
        im = {
            "xT": xT, "fcwb": fcwb.astype(np.float32),
            "srcs": idx_tile, "selc": scq, "selt": stq,
            "dstids": np.ascontiguousarray(dids),
            "pmask": pm, "idnt": idnt,
            "pmrep": np.ascontiguousarray(
                np.broadcast_to(pm, (P, NLOC))).astype(ml_dtypes.bfloat16),
            "_didx": dst_slot[k].astype(np.int64).reshape(TS, P).T,
        }
        for l in (1, 2, 3):
            im[f"wc{l}"] = wcs[l - 1]
            im[f"bng{l}"] = bngs[l - 1]
            im[f"bnb{l}"] = bnbs[l - 1]
        in_maps.append(im)
    return SEGLEN, tuple(int(v) for v in nsubs), N, in_maps


_CACHE = {}


def _get_nc(SEGLEN, NSUB, n_real):
    key = (SEGLEN, NSUB, n_real)
    if key not in _CACHE:
        nc = build_gnn(SEGLEN, NSUB, n_real)
        hoist_excess_waits(nc)
        _CACHE[key] = nc
    return _CACHE[key]


def kernel(**inputs):
    patch_walrus()
    SEGLEN, NSUB, n_real, in_maps = prepare(inputs)
    nc = _get_nc(SEGLEN, NSUB, n_real)
    from concourse import bass_utils
    res = bass_utils.run_bass_kernel_spmd(
        nc, in_maps, core_ids=list(range(NCORES)))
    return np.asarray(res.results[0]["out"]).astype(np.float32)


# revision 27
# speedup vs baseline: 1.6754x; 1.0220x over previous
"""Trainium2 Bass kernel for 3-layer GAT + BN/ReLU + global max pool + FC.

v2. Sharding: nodes renumbered so each of the 16 graphs (batch segments) gets
a fixed padded block of SEGLEN nodes; core k owns segments {2k,2k+1} (NLOC
contiguous device nodes). Edges partitioned by destination core, grouped in
128-dst windows; segmented softmax-sum + feature aggregation are PE matmuls
against host-precomputed 0/1 selector matrices (selc input, SBUF-resident).
Features are stored c-major (col = c*H+h) so the per-edge alpha multiply hits
the DVE 2x perf mode. Per-edge source rows come from one dma_gather per
window out of a per-layer DRAM table [h bf16 | als]; per-edge dst alpha from
one batched 8-elem dma_gather per layer out of aldtab. Params replicated; two
chunked AllGathers per layer carry channel-major pre-BN output (chunk B also
carries hi/lo-packed BN partial sums) so network overlaps edge compute.
"""
import sys

sys.path.insert(0, "/opt/trn_rl_repo")
sys.path.insert(0, "/opt/trn_rl_repo/concourse")

import numpy as np
import ml_dtypes

import concourse.bass as bass
import concourse.tile as tile
import concourse.mybir as mybir
from concourse import bacc
from concourse.bass import IndirectOffsetOnAxis

P = 128
NCORES = 8
H = 8
EPS_BN = 1e-5
NEG_SLOPE = 0.2
F32 = mybir.dt.float32
BF16 = mybir.dt.bfloat16
FP8 = mybir.dt.float8e4

CINS = [3, 128, 256]
CS = [16, 32, 64]
HCS = [128, 256, 512]
ROWES = [256, 384, 640]   # table row stride in bf16 elems (256B units)
WRITEWS = [144, 272, 528]  # written row prefix: h (c-major) + als + ald
NTILES = [1, 2, 4]        # HC/128 per layer
NKS = [1, 1, 2]           # K-tiles per dense matmul
GRP = 8                   # table-write batching (tiles per DMA)

AX = mybir.AxisListType
ALU = mybir.AluOpType
ACTF = mybir.ActivationFunctionType


def apx(sl, dims):
    """AP with custom free dims [(step, count), ...] on a tile slice's base."""
    return bass.AP(sl.tensor, sl.offset,
                   [list(sl.ap[0])] + [list(d) for d in dims])


def apd(t, off, dims):
    """Raw AP on a DRAM tensor with explicit dims."""
    return bass.AP(t, off, [list(d) for d in dims])


def hoist_excess_waits(nc, cap=4):
    cnt = [0]
    total = 0
    for f in nc.m.functions:
        for b in f.blocks:
            if not any(
                i.sync_info is not None and i.sync_info.on_wait
                and len(i.sync_info.on_wait) > cap
                for i in b.instructions
            ):
                continue
            new_insts = []
            for inst in b.instructions:
                si = inst.sync_info
                if si is not None and si.on_wait and len(si.on_wait) > cap:
                    waits = list(si.on_wait)
                    for w in waits[:-cap]:
                        cnt[0] += 1
                        new_insts.append(
                            mybir.InstNoOp(
                                name=f"hw-{cnt[0]}",
                                engine=inst.engine,
                                ins=[],
                                outs=[],
                                sync_info=mybir.SyncInfo(on_wait=[w],
                                                         on_update=[]),
                            )
                        )
                        total += 1
                    inst.sync_info = mybir.SyncInfo(
                        on_wait=waits[-cap:], on_update=list(si.on_update))
                new_insts.append(inst)
            b.instructions = new_insts
    return total


def patch_walrus():
    from concourse import bass_utils as bu
    if getattr(bu, "_dge_patched", False):
        return
    orig = bu.get_walrus_args

    def patched(*a, **k):
        return orig(*a, **k) + [
            "--dge-levels=io,spill_reload,scalar_dynamic_offset,"
            "vector_dynamic_offsets,dst_reduce"
        ]

    bu.get_walrus_args = patched
    bu._dge_patched = True


def build_gnn(SEGLEN, NSUB, n_real, stages=99, repeat=1, dbg=False):
    nsubs = list(NSUB) if isinstance(NSUB, (list, tuple)) else None
    NLOC = 2 * SEGLEN
    NW = NLOC // P
    NPD = 16 * SEGLEN
    SEGW = NW // 2
    if nsubs is None:
        nsubs = [NSUB] * NW
    csum = [0]
    for v in nsubs:
        csum.append(csum[-1] + v)
    TS = csum[-1]  # total subchunks per core
    NSMAX = max(nsubs)
    CHW = NW // 2            # windows in AllGather chunk A
    CAC = CHW * P            # chunk A columns
    CBC = NLOC - CAC + 4     # chunk B columns (incl packed stats)

    nc = bacc.Bacc("TRN2", target_bir_lowering=False, debug=False,
                   num_devices=NCORES, num_swdge_queues=2)

    # ---------------- I/O ----------------
    xT = nc.dram_tensor("xT", [3, NPD], BF16, kind="ExternalInput")
    wc = [nc.dram_tensor(f"wc{l}", [P if l > 1 else 3,
                                    NKS[l - 1] * (HCS[l - 1] + 16)], BF16,
                         kind="ExternalInput") for l in (1, 2, 3)]
    bng = [nc.dram_tensor(f"bng{l}", [P, NTILES[l - 1]], F32,
                          kind="ExternalInput") for l in (1, 2, 3)]
    bnb = [nc.dram_tensor(f"bnb{l}", [P, NTILES[l - 1]], F32,
                          kind="ExternalInput") for l in (1, 2, 3)]
    fcwb = nc.dram_tensor("fcwb", [513, 10], F32, kind="ExternalInput")
    srcs = nc.dram_tensor("srcs", [P, TS * 8], mybir.dt.int16,
                          kind="ExternalInput")
    dstids = nc.dram_tensor("dstids", [P, NW], mybir.dt.int32,
                            kind="ExternalInput")
    selc = nc.dram_tensor("selc", [P, TS * P], FP8, kind="ExternalInput")
    selt = nc.dram_tensor("selt", [P, TS * P], FP8, kind="ExternalInput")
    pmask = nc.dram_tensor("pmask", [1, NLOC], F32, kind="ExternalInput")
    pmrep = nc.dram_tensor("pmrep", [P, NLOC], BF16, kind="ExternalInput")
    idnt = nc.dram_tensor("idnt", [P, P], F32, kind="ExternalInput")
    out = nc.dram_tensor("out", [16, 10], F32, kind="ExternalOutput")

    # ---------------- internal DRAM ----------------
    tables = [nc.dram_tensor(f"table{l}", [NPD, ROWES[l - 1]], BF16)
              for l in (1, 2, 3)]
    agA_in = [nc.dram_tensor(f"agA{l}_in", [NTILES[l - 1] * P, CAC], BF16)
              for l in (1, 2)]
    agA_out = [nc.dram_tensor(f"agA{l}_out",
                              [NCORES * NTILES[l - 1] * P, CAC], BF16,
                              addr_space="Shared")
               for l in (1, 2)]
    agB_in = [nc.dram_tensor(f"agB{l}_in", [NTILES[l - 1] * P, CBC], BF16)
              for l in (1, 2)]
    agB_out = [nc.dram_tensor(f"agB{l}_out",
                              [NCORES * NTILES[l - 1] * P, CBC], BF16,
                              addr_space="Shared")
               for l in (1, 2)]
    if dbg:
        dbgtab = nc.dram_tensor("dbgtab", [NPD, ROWES[0]], BF16,
                                kind="ExternalOutput")
        dbgz = nc.dram_tensor("dbgz", [P, CAC + CBC - 4], BF16,
                              kind="ExternalOutput")
    st3_in = nc.dram_tensor("st3_in", [4 * P, 4], F32)
    st3_out = nc.dram_tensor("st3_out", [NCORES * 4 * P, 4], F32,
                             addr_space="Shared")

    RG = [list(range(NCORES))]

    with tile.TileContext(nc) as tc, tc.tile_pool(name="const", bufs=1) as cp:
        # ---------- constants ----------
        ident = cp.tile([P, P], F32, tag="idnt", name="idnt")
        nc.sync.dma_start(ident[:], idnt.ap())
        onesf = cp.tile([1, P], F32, tag="onesf", name="onesf")
        nc.vector.memset(onesf[:], 1.0)
        ones16 = cp.tile([1, 16], F32, tag="ones16", name="ones16")
        nc.vector.memset(ones16[:], 1.0)

        wc_t = []
        for i in range(3):
            t = cp.tile([P if i > 0 else 3, NKS[i] * (HCS[i] + 16)], BF16,
                        tag=f"wc{i}", name=f"wc{i}")
            nc.sync.dma_start(t[:], wc[i].ap())
            wc_t.append(t)
        bng_t, bnb_t = [], []
        for i in range(3):
            tg = cp.tile([P, NTILES[i]], F32, tag=f"bng{i}", name=f"bng{i}")
            tb = cp.tile([P, NTILES[i]], F32, tag=f"bnb{i}", name=f"bnb{i}")
            nc.sync.dma_start(tg[:], bng[i].ap())
            nc.sync.dma_start(tb[:], bnb[i].ap())
            bng_t.append(tg)
            bnb_t.append(tb)
        fcw_t = []
        for c in range(4):
            t = cp.tile([P, 10], F32, tag=f"fcw{c}", name=f"fcw{c}")
            nc.sync.dma_start(t[:], fcwb.ap()[c * P:(c + 1) * P, :])
            fcw_t.append(t)
        fcb_t = cp.tile([1, 10], F32, tag="fcb", name="fcb")
        nc.sync.dma_start(fcb_t[:], fcwb.ap()[512:513, :])
        pmask_t = cp.tile([1, NLOC], F32, tag="pmask", name="pmask")
        nc.sync.dma_start(pmask_t[:], pmask.ap())
        pmrep_t = cp.tile([P, NLOC], BF16, tag="pmrep", name="pmrep")
        nc.sync.dma_start(pmrep_t[:], pmrep.ap())
        idx_t = cp.tile([P, TS * 8], mybir.dt.int16, tag="idx", name="idx")
        nc.sync.dma_start(idx_t[:], srcs.ap())
        dids_t = cp.tile([P, NW], mybir.dt.int32, tag="dids", name="dids")
        nc.sync.dma_start(dids_t[:], dstids.ap())

        def compute_AB(pool, s12, gi, c):
            mu = pool.tile([P, 1], F32, tag="mu", name="mu")
            nc.vector.tensor_scalar(mu[:], s12[:, 0:1], 1.0 / n_real, None,
                                    op0=ALU.mult)
            ex2 = pool.tile([P, 1], F32, tag="ex2", name="ex2")
            nc.vector.tensor_scalar(ex2[:], s12[:, 1:2], 1.0 / n_real, None,
                                    op0=ALU.mult)
            var = pool.tile([P, 1], F32, tag="var", name="var")
            nc.vector.tensor_tensor(out=var[:], in0=mu[:], in1=mu[:],
                                    op=ALU.mult)
            nc.vector.tensor_tensor(out=var[:], in0=ex2[:], in1=var[:],
                                    op=ALU.subtract)
            nc.vector.tensor_scalar(var[:], var[:], EPS_BN, None, op0=ALU.add)
            sd = pool.tile([P, 1], F32, tag="sd", name="sd")
            nc.scalar.sqrt(sd[:], var[:])
            rs = pool.tile([P, 1], F32, tag="rs", name="rs")
            nc.vector.reciprocal(rs[:], sd[:])
            A = pool.tile([P, 1], F32, tag="A", name="A")
            nc.vector.tensor_tensor(out=A[:], in0=rs[:],
                                    in1=bng_t[gi][:, c:c + 1], op=ALU.mult)
            B = pool.tile([P, 1], F32, tag="B", name="B")
            nc.vector.tensor_tensor(out=B[:], in0=mu[:], in1=A[:],
                                    op=ALU.mult)
            nc.vector.tensor_tensor(out=B[:], in0=bnb_t[gi][:, c:c + 1],
                                    in1=B[:], op=ALU.subtract)
            return A, B

        def dense_phase(l):
            """h = x @ W for all NPD rows (replicated on every core).

            Writes [h(c-major) | als] prefix of each table row, plus aldtab.
            For l>=2, x is read per-core-block from the layer-(l-1) AllGather
            chunks with BN+ReLU applied on the fly.
            """
            li = l - 1
            HC, ROWE, WW = HCS[li], ROWES[li], WRITEWS[li]
            nk = NKS[li]
            WCW = HC + 16
            fused = WCW <= 512
            with (
                tc.tile_pool(name=f"d{l}", bufs=3) as dp,
                tc.tile_pool(name=f"dx{l}", bufs=2) as xp,
                tc.tile_pool(name=f"dps{l}", bufs=3, space="PSUM") as pp,
            ):
                AB = None
                if l > 1:
                    pli = li - 1
                    pntile = NTILES[pli]
                    pCBC = CBC
                    AB = []
                    for c in range(pntile):
                        stt = dp.tile([P, 8, 4], BF16, tag="stt", name="stt")
                        nc.sync.dma_start(
                            stt[:],
                            apd(agB_out[pli], c * P * pCBC + (pCBC - 4),
                                [[pCBC, P], [pntile * P * pCBC, 8], [1, 4]]))
                        acc = dp.tile([P, 2], F32, tag="acc", name="acc")
                        nc.vector.memset(acc[:], 0.0)
                        for k in range(NCORES):
                            nc.vector.tensor_tensor(
                                out=acc[:], in0=acc[:], in1=stt[:, k, 0:2],
                                op=ALU.add)
                            nc.vector.tensor_tensor(
                                out=acc[:], in0=acc[:], in1=stt[:, k, 2:4],
                                op=ALU.add)
                        AB.append(compute_AB(dp, acc, li - 1, c))

                for k in range(NCORES):
                    if l == 1:
                        xts = None
                    else:
                        pli = li - 1
                        pntile = NTILES[pli]
                        xts = []
                        for c in range(nk):
                            blk = xp.tile([P, NLOC], BF16, tag=f"blk{c}",
                                          name=f"blk{c}")
                            r0 = (k * pntile + c) * P
                            nc.sync.dma_start(
                                blk[:, 0:CAC],
                                agA_out[pli].ap()[r0:r0 + P, :])
                            nc.sync.dma_start(
                                blk[:, CAC:NLOC],
                                agB_out[pli].ap()[r0:r0 + P, 0:NLOC - CAC])
                            xt = xp.tile([P, NLOC], BF16, tag=f"x{c}",
                                         name=f"x{c}")
                            A, B = AB[c]
                            nc.scalar.activation(xt[:], blk[:], ACTF.Relu,
                                                 bias=B[:], scale=A[:])
                            xts.append(xt)
                    stage = None
                    for jj in range(NW):
                        gi = jj % GRP
                        if gi == 0:
                            ng = min(GRP, NW - jj)
                            stage = dp.tile([P, GRP, ROWE], BF16, tag="stage",
                                            name="stage")
                        psA = pp.tile([P, WCW if fused else HC], F32,
                                      tag="psA", name="psA")
                        psB = None if fused else pp.tile([P, 16], F32,
                                                         tag="psB", name="psB")
                        for c in range(nk):
                            if l == 1:
                                lhs = xT_t[:, k * NLOC + jj * P:
                                           k * NLOC + jj * P + P]
                            else:
                                lhs = xts[c][:, jj * P:(jj + 1) * P]
                            if fused:
                                nc.tensor.matmul(
                                    out=psA[:], lhsT=lhs,
                                    rhs=wc_t[li][:, c * WCW:(c + 1) * WCW],
                                    start=(c == 0), stop=(c == nk - 1))
                            else:
                                rhsW = wc_t[li][:, c * WCW:c * WCW + HC]
                                rhsb = wc_t[li][:, c * WCW + HC:(c + 1) * WCW]
                                nc.tensor.matmul(out=psA[:], lhsT=lhs,
                                                 rhs=rhsW, start=(c == 0),
                                                 stop=(c == nk - 1))
                                nc.tensor.matmul(out=psB[:], lhsT=lhs,
                                                 rhs=rhsb, start=(c == 0),
                                                 stop=(c == nk - 1))
                        def cpy(dst, src, _a=(jj % 2 == 1)):
                            if _a:
                                nc.scalar.copy(dst, src)
                            else:
                                nc.vector.tensor_copy(dst, src)
                        if fused:
                            cpy(stage[:, gi, 0:WW], psA[:, 0:WW])
                        else:
                            cpy(stage[:, gi, 0:HC], psA[:])
                            cpy(stage[:, gi, HC:HC + 16], psB[:])
                        if gi == ng - 1:
                            base = k * NLOC + (jj - gi) * P
                            nc.sync.dma_start(
                                bass.AP(tables[li], base * ROWE,
                                        [[ROWE, P], [P * ROWE, ng], [1, WW]]),
                                apx(stage[:], [[ROWE, ng], [1, WW]]))

        def edge_phase(l, selc_t, selt_t):
            """Software-pipelined window loop (depth PD).

            Stage A(w): gather g, psew matmuls, ew chain -> wbf.
            Stage B(w): wha, psf/psd matmuls, softmax div, transpose, zT/stats.
            For l==3 the selector tiles are streamed from DRAM per window.
            """
            li = l - 1
            HC, C, ROWE = HCS[li], CS[li], ROWES[li]
            ntile = NTILES[li]
            PD = 2
            stream_sel = selc_t is None
            with (
                tc.tile_pool(name=f"e{l}", bufs=PD + 2) as ep,
                tc.tile_pool(name=f"ew{l}", bufs=2) as wp,
                tc.tile_pool(name=f"eg{l}", bufs=PD + 1) as gp,
                tc.tile_pool(name=f"es{l}", bufs=PD + 1) as slp,
                tc.tile_pool(name=f"ez{l}", bufs=1) as zp,
                tc.tile_pool(name=f"eps{l}", bufs=2, space="PSUM") as pp,
                tc.tile_pool(name=f"epw{l}", bufs=2, space="PSUM") as ppw,
                tc.tile_pool(name=f"epf{l}", bufs=2, space="PSUM") as ppf,
            ):
                if dbg and l == 1:
                    nc.sync.dma_start(dbgtab.ap(), tables[0].ap())
                aldbf_all = zp.tile([P, NW, 8], BF16, tag="aldbf",
                                    name="aldbf")
                for w in range(NW):
                    nc.gpsimd.indirect_dma_start(
                        out=aldbf_all[:, w, :],
                        out_offset=None,
                        in_=tables[li].ap(),
                        in_offset=IndirectOffsetOnAxis(
                            ap=dids_t[:, w:w + 1], axis=0),
                        element_offset=HC + 8,
                    )
                zTA = [zp.tile([P, CAC], BF16, tag=f"zTA{c}",
                               name=f"zTA{c}") for c in range(ntile)]
                zTB = [zp.tile([P, CBC if l < 3 else NLOC - CAC], BF16,
                               tag=f"zTB{c}", name=f"zTB{c}")
                       for c in range(ntile)]

                live = {}

                def stage_a(w):
                    ns = nsubs[w]
                    base = csum[w]
                    g = gp.tile([P, NSMAX, ROWE], BF16, tag="g",
                                name="g")[:, 0:ns, :]
                    nc.gpsimd.dma_gather(
                        out_ap=g[:],
                        in_ap=tables[li].ap(),
                        idxs_ap=idx_t[:, base * 8:(base + ns) * 8],
                        num_idxs=ns * P,
                        num_idxs_reg=ns * P,
                        elem_size=ROWE,
                        single_packet=False,
                        queue_num=w % 2,
                    )
                    if stream_sel:
                        scw = slp.tile([P, NSMAX * P], FP8, tag="scw",
                                       name="scw")[:, 0:ns * P]
                        nc.sync.dma_start(
                            scw[:], selc.ap()[:, base * P:(base + ns) * P])
                        stw = slp.tile([P, NSMAX * P], FP8, tag="stw",
                                       name="stw")[:, 0:ns * P]
                        nc.sync.dma_start(
                            stw[:], selt.ap()[:, base * P:(base + ns) * P])
                    else:
                        scw = selc_t[:, base * P:(base + ns) * P]
                        stw = selt_t[:, base * P:(base + ns) * P]
                    psew = ppw.tile([P, NSMAX * 8], F32, tag="psew",
                                    name="psew")[:, 0:ns * 8]
                    for s in range(ns):
                        nc.tensor.matmul(
                            out=psew[:, s * 8:(s + 1) * 8],
                            lhsT=stw[:, s * P:(s + 1) * P],
                            rhs=aldbf_all[:, w, :], start=True, stop=True)
                    ew = ep.tile([P, NSMAX, 8], BF16, tag="ew",
                                 name="ew")[:, 0:ns, :]
                    nc.vector.tensor_tensor(
                        out=ew[:],
                        in0=apx(g[:, 0, HC:HC + 8], [(ROWE, ns), (1, 8)]),
                        in1=apx(psew[:], [(8, ns), (1, 8)]),
                        op=ALU.add)
                    ew2 = ep.tile([P, NSMAX, 8], BF16, tag="ew2",
                                  name="ew2")[:, 0:ns, :]
                    nc.vector.tensor_scalar(ew2[:], ew[:], NEG_SLOPE, None,
                                            op0=ALU.mult)
                    nc.vector.tensor_tensor(out=ew2[:], in0=ew[:], in1=ew2[:],
                                            op=ALU.max)
                    wbf = ep.tile([P, NSMAX, 8], BF16, tag="wbf",
                                  name="wbf")[:, 0:ns, :]
                    nc.scalar.activation(wbf[:], ew2[:], ACTF.Exp)
                    live[w] = (g, scw, wbf)

                def stage_b(w):
                    ns = nsubs[w]
                    base = csum[w]
                    g, scw, wbf = live.pop(w)
                    merged = HC + 8 <= 512
                    psf = ppf.tile([P, HC + 8 if merged else HC], F32,
                                   tag="psf", name="psf")
                    psd = None if merged else pp.tile([P, 8], F32, tag="psd",
                                                      name="psd")
                    WHW = HC + 8 if merged else HC
                    wha = wp.tile([P, NSMAX, WHW], BF16, tag="wha",
                                  name="wha")[:, 0:ns, :]
                    nc.vector.tensor_tensor(
                        out=apx(wha[:], [(WHW, ns), (H, C), (1, H)]),
                        in0=apx(g[:, 0, 0:HC], [(ROWE, ns), (H, C), (1, H)]),
                        in1=apx(wbf[:, 0, :], [(8, ns), (0, C), (1, 8)]),
                        op=ALU.mult)
                    if merged:
                        nc.vector.tensor_copy(
                            apx(wha[:, 0, HC:HC + 8], [(WHW, ns), (1, 8)]),
                            apx(wbf[:, 0, :], [(8, ns), (1, 8)]))
                    for s in range(ns):
                        sl = scw[:, s * P:(s + 1) * P]
                        nc.tensor.matmul(
                            out=psf[:], lhsT=sl, rhs=wha[:, s, :],
                            start=(s == 0), stop=(s == ns - 1))
                        if not merged:
                            nc.tensor.matmul(
                                out=psd[:], lhsT=sl, rhs=wbf[:, s, :],
                                start=(s == 0), stop=(s == ns - 1))

                    den = ep.tile([P, 8], F32, tag="den", name="den")
                    nc.vector.tensor_scalar(
                        den[:], psf[:, HC:HC + 8] if merged else psd[:],
                        1e-16, None, op0=ALU.add)
                    rec = ep.tile([P, 8], F32, tag="rec", name="rec")
                    nc.vector.reciprocal(rec[:], den[:])
                    z = ep.tile([P, HC], F32, tag="z", name="z")
                    nc.vector.tensor_tensor(
                        out=apx(z[:], [(H, C), (1, H)]),
                        in0=apx(psf[:], [(H, C), (1, H)]),
                        in1=apx(rec[:], [(0, C), (1, H)]),
                        op=ALU.mult)

                    for c in range(ntile):
                        pt = pp.tile([P, P], F32, tag="pt", name="pt")
                        nc.tensor.transpose(pt[:], z[:, c * P:(c + 1) * P],
                                            ident[:])
                        if w < CHW:
                            nc.scalar.copy(
                                zTA[c][:, w * P:(w + 1) * P], pt[:])
                        else:
                            nc.scalar.copy(
                                zTB[c][:, (w - CHW) * P:
                                       (w - CHW + 1) * P], pt[:])

                    if l < 3 and w == CHW - 1:
                        for c in range(ntile):
                            nc.sync.dma_start(
                                agA_in[li].ap()[c * P:(c + 1) * P, :],
                                zTA[c][:])
                        nc.gpsimd.collective_compute(
                            "AllGather", ALU.bypass, replica_groups=RG,
                            ins=[agA_in[li].ap().opt()],
                            outs=[agA_out[li].ap().opt()])

                for w in range(NW + PD):
                    if w < NW:
                        stage_a(w)
                    if w >= PD:
                        stage_b(w - PD)

                if l < 3:
                    for c in range(ntile):
                        s1 = ep.tile([P, 1], F32, tag="s1", name="s1")
                        s1b = ep.tile([P, 1], F32, tag="s1b", name="s1b")
                        nc.vector.reduce_sum(s1[:], zTA[c][:], axis=AX.X)
                        nc.vector.reduce_sum(s1b[:], zTB[c][:, 0:NLOC - CAC],
                                             axis=AX.X)
                        scr = ep.tile([P, CAC], BF16, tag="scr", name="scr")
                        s2 = ep.tile([P, 1], F32, tag="s2", name="s2")
                        s2b = ep.tile([P, 1], F32, tag="s2b", name="s2b")
                        nc.scalar.activation(scr[:], zTA[c][:], ACTF.Square,
                                             accum_out=s2[:])
                        nc.scalar.activation(scr[:, 0:NLOC - CAC],
                                             zTB[c][:, 0:NLOC - CAC],
                                             ACTF.Square, accum_out=s2b[:])
                        s12t = ep.tile([P, 2], F32, tag="s12t", name="s12t")
                        nc.vector.tensor_tensor(out=s12t[:, 0:1], in0=s1[:],
                                                in1=s1b[:], op=ALU.add)
                        nc.vector.tensor_tensor(out=s12t[:, 1:2], in0=s2[:],
                                                in1=s2b[:], op=ALU.add)
                        # pack fp32 sums as bf16 hi/lo pairs
                        nc.vector.tensor_copy(zTB[c][:, NLOC - CAC:
                                                     NLOC - CAC + 2],
                                              s12t[:])
                        hif = ep.tile([P, 2], F32, tag="hif", name="hif")
                        nc.vector.tensor_copy(hif[:],
                                              zTB[c][:, NLOC - CAC:
                                                     NLOC - CAC + 2])
                        lo = ep.tile([P, 2], F32, tag="lo", name="lo")
                        nc.vector.tensor_tensor(out=lo[:], in0=s12t[:],
                                                in1=hif[:], op=ALU.subtract)
                        nc.vector.tensor_copy(zTB[c][:, NLOC - CAC + 2:
                                                     NLOC - CAC + 4], lo[:])
                        nc.sync.dma_start(
                            agB_in[li].ap()[c * P:(c + 1) * P, :], zTB[c][:])
                    nc.gpsimd.collective_compute(
                        "AllGather", ALU.bypass, replica_groups=RG,
                        ins=[agB_in[li].ap().opt()],
                        outs=[agB_out[li].ap().opt()])
                    return
                # ---------- layer-3 strip stats + tail ----------
                s12 = []
                sgm = []
                for c in range(ntile):
                    s1 = ep.tile([P, 1], F32, tag="s1", name="s1")
                    s1b = ep.tile([P, 1], F32, tag="s1b", name="s1b")
                    nc.vector.reduce_sum(s1[:], zTA[c][:], axis=AX.X)
                    nc.vector.reduce_sum(s1b[:], zTB[c][:], axis=AX.X)
                    scr = ep.tile([P, CAC], BF16, tag="scr", name="scr")
                    s2 = ep.tile([P, 1], F32, tag="s2", name="s2")
                    s2b = ep.tile([P, 1], F32, tag="s2b", name="s2b")
                    nc.scalar.activation(scr[:], zTA[c][:], ACTF.Square,
                                         accum_out=s2[:])
                    nc.scalar.activation(scr[:, 0:NLOC - CAC], zTB[c][:],
                                         ACTF.Square, accum_out=s2b[:])
                    s12c = ep.tile([P, 2], F32, tag="s12c", name="s12c")
                    nc.vector.tensor_tensor(out=s12c[:, 0:1], in0=s1[:],
                                            in1=s1b[:], op=ALU.add)
                    nc.vector.tensor_tensor(out=s12c[:, 1:2], in0=s2[:],
                                            in1=s2b[:], op=ALU.add)
                    s12.append(s12c)
                    mz = ep.tile([P, CAC], BF16, tag="mz", name="mz")
                    sgmc = ep.tile([P, 2], F32, tag="sgmc", name="sgmc")
                    nc.vector.tensor_tensor(
                        out=mz[:], in0=zTA[c][:],
                        in1=apx(pmrep_t[:, 0:CAC], [(1, CAC)]), op=ALU.add)
                    nc.vector.reduce_max(sgmc[:, 0:1], mz[:], axis=AX.X)
                    nc.vector.tensor_tensor(
                        out=mz[:, 0:NLOC - CAC], in0=zTB[c][:],
                        in1=apx(pmrep_t[:, CAC:NLOC], [(1, NLOC - CAC)]),
                        op=ALU.add)
                    nc.vector.reduce_max(sgmc[:, 1:2], mz[:, 0:NLOC - CAC],
                                         axis=AX.X)
                    sgm.append(sgmc)
                for c in range(ntile):
                    pk = ep.tile([P, 4], F32, tag="pk", name="pk")
                    nc.vector.tensor_copy(pk[:, 0:2], s12[c][:])
                    nc.vector.tensor_copy(pk[:, 2:4], sgm[c][:])
                    nc.sync.dma_start(st3_in.ap()[c * P:(c + 1) * P, :],
                                      pk[:])
                nc.gpsimd.collective_compute(
                    "AllGather", ALU.bypass, replica_groups=RG,
                    ins=[st3_in.ap().opt()], outs=[st3_out.ap().opt()])
                with tc.tile_pool(name="tail", bufs=2) as tp:
                    psfc = pp.tile([16, 10], F32, tag="pt", name="psfc")
                    for c in range(4):
                        stt = tp.tile([P, 8, 4], F32, tag="st3t", name="st3t")
                        nc.sync.dma_start(
                            stt[:],
                            bass.AP(st3_out, c * P * 4,
                                    [[4, P], [4 * P * 4, 8], [1, 4]]))
                        acc = tp.tile([P, 2], F32, tag="stacc", name="stacc")
                        nc.vector.memset(acc[:], 0.0)
                        for k in range(NCORES):
                            nc.vector.tensor_tensor(
                                out=acc[:], in0=acc[:], in1=stt[:, k, 0:2],
                                op=ALU.add)
                        A, B = compute_AB(tp, acc, 2, c)
                        pooled = tp.tile([P, 16], F32, tag="pooled",
                                         name="pooled")
                        for k in range(NCORES):
                            nc.scalar.activation(pooled[:, 2 * k:2 * k + 2],
                                                 stt[:, k, 2:4], ACTF.Relu,
                                                 bias=B[:], scale=A[:])
                        nc.tensor.matmul(out=psfc[:], lhsT=pooled[:],
                                         rhs=fcw_t[c][:], start=(c == 0),
                                         stop=False, skip_group_check=True)
                    nc.tensor.matmul(out=psfc[:], lhsT=ones16[:],
                                     rhs=fcb_t[:], start=False, stop=True,
                                     skip_group_check=True)
                    ot = tp.tile([16, 10], F32, tag="ot", name="ot")
                    nc.vector.tensor_copy(ot[:], psfc[:])
                    nc.sync.dma_start(out.ap(), ot[:])

        for _rep in range(repeat):
            selp = tc.tile_pool(name="selp", bufs=1)
            sp = selp.__enter__()
            selc_t = sp.tile([P, TS * P], FP8, tag="selc", name="selc")
            nc.sync.dma_start(selc_t[:], selc.ap())
            selt_t = sp.tile([P, TS * P], FP8, tag="selt", name="selt")
            nc.sync.dma_start(selt_t[:], selt.ap())
            xp1 = tc.tile_pool(name="x1", bufs=1)
            xpool1 = xp1.__enter__()
            xT_t = xpool1.tile([3, NPD], BF16, tag="xT", name="xT")
            nc.sync.dma_start(xT_t[:], xT.ap())
            if stages >= 1:
                dense_phase(1)
            xp1.__exit__(None, None, None)
            if stages >= 2:
                edge_phase(1, selc_t, selt_t)
            if stages >= 3:
                dense_phase(2)
            if stages >= 4:
                edge_phase(2, selc_t, selt_t)
            selp.__exit__(None, None, None)
            if stages >= 5:
                dense_phase(3)
            if stages >= 6:
                edge_phase(3, None, None)

    nc.compile()
    return nc


# ================= host preprocessing =================

def _cmajor_idx(C):
    """idx[c*H+h] = h*C + c  (c-major column order for [C,H] heads layout)."""
    return (np.arange(H)[None, :] * C + np.arange(C)[:, None]).ravel()


def prepare(inputs):
    x = np.asarray(inputs["x"], np.float32)
    ei = np.asarray(inputs["edge_index"])
    batch = np.asarray(inputs["batch"]).astype(np.int64)
    N = x.shape[0]
    assert np.all(np.diff(batch) >= 0), "batch must be sorted"
    seg_sizes = np.bincount(batch, minlength=16)
    SEGLEN = int(np.ceil(max(seg_sizes.max(), 1) / P) * P)
    NLOC = 2 * SEGLEN
    NW = NLOC // P
    NPD = 16 * SEGLEN
    assert NPD < 32768, "device node ids must fit int16 for dma_gather"
    seg_start = np.zeros(16, np.int64)
    seg_start[1:] = np.cumsum(seg_sizes)[:-1]
    dev_of = batch * SEGLEN + (np.arange(N) - seg_start[batch])

    src = np.concatenate([ei[0].astype(np.int64), np.arange(N)])
    dst = np.concatenate([ei[1].astype(np.int64), np.arange(N)])
    sdev = dev_of[src]
    ddev = dev_of[dst]
    core = ddev // NLOC
    dloc = ddev % NLOC
    win = dloc // P
    wloc = dloc % P
    key = core * NW + win
    counts = np.bincount(key, minlength=NCORES * NW)
    cw = counts.reshape(NCORES, NW)
    nsubs = np.maximum(1, np.ceil(cw.max(axis=0) / P).astype(np.int64))
    csum = np.zeros(NW + 1, np.int64)
    csum[1:] = np.cumsum(nsubs)
    TS = int(csum[-1])

    perm = np.argsort(key, kind="stable")
    gstart = np.zeros(NCORES * NW, np.int64)
    gstart[1:] = np.cumsum(counts)[:-1]
    pos = np.arange(len(perm)) - gstart[key[perm]]
    kperm = key[perm]
    wbase = (csum[:-1] * P)[kperm % NW]
    slot = (kperm // NW) * (TS * P) + wbase + pos

    src_slot = np.zeros(NCORES * TS * P, np.int16)
    dst_slot = np.zeros(NCORES * TS * P, np.int16)
    dl_slot = np.full(NCORES * TS * P, 300, np.int64)
    src_slot[slot] = sdev[perm].astype(np.int16)
    dst_slot[slot] = ddev[perm].astype(np.int16)
    dl_slot[slot] = wloc[perm]
    src_slot = src_slot.reshape(NCORES, TS * P)
    dst_slot = dst_slot.reshape(NCORES, TS * P)
    dl_slot = dl_slot.reshape(NCORES, TS * P)

    wcs, bngs, bnbs = [], [], []
    prev_idx = None
    for l, (cin, C) in enumerate([(3, 16), (128, 32), (256, 64)], start=1):
        W = np.asarray(inputs[f"W{l}"], np.float32)
        a_s = np.asarray(inputs[f"as{l}"], np.float32)
        a_d = np.asarray(inputs[f"ad{l}"], np.float32)
        HC = H * C
        idx = _cmajor_idx(C)
        Asm = np.zeros((HC, H), np.float32)
        Adm = np.zeros((HC, H), np.float32)
        for hd in range(H):
            Asm[hd * C:(hd + 1) * C, hd] = a_s[hd]
            Adm[hd * C:(hd + 1) * C, hd] = a_d[hd]
        if prev_idx is not None:
            W = W[prev_idx, :]
        wcat = np.concatenate([W[:, idx], W @ Asm, W @ Adm], axis=1)
        nk = NKS[l - 1]
        if nk > 1:
            wcat = np.concatenate(
                [wcat[c * P:(c + 1) * P] for c in range(nk)], axis=1)
        wcs.append(np.ascontiguousarray(wcat).astype(ml_dtypes.bfloat16))
        nt = HC // P
        bngs.append(np.ascontiguousarray(
            np.asarray(inputs[f"g{l}"], np.float32)[idx].reshape(nt, P).T))
        bnbs.append(np.ascontiguousarray(
            np.asarray(inputs[f"be{l}"], np.float32)[idx].reshape(nt, P).T))
        prev_idx = idx
    fcwb = np.concatenate(
        [np.asarray(inputs["fcW"], np.float32)[prev_idx, :],
         np.asarray(inputs["fcb"], np.float32)[None, :]], axis=0)

    x_dev = np.zeros((NPD, 3), np.float32)
    x_dev[dev_of] = x
    xT = np.ascontiguousarray(x_dev.T).astype(ml_dtypes.bfloat16)

    idnt = np.eye(P, dtype=np.float32)

    in_maps = []
    for k in range(NCORES):
        def pack16(v16):
            w16 = v16.reshape(TS * 8, 16).T
            t = np.zeros((P, TS * 8), np.int16)
            t[:16] = w16
            t[16:] = np.tile(w16, (7, 1))
            return t
        idx_tile = pack16(src_slot[k])
        dl = dl_slot[k]
        sc = np.zeros((TS * P, P), np.float32)
        valid = dl < P
        sc[np.nonzero(valid)[0], dl[valid]] = 1.0
        M = sc.reshape(TS, P, P)
        scq = np.ascontiguousarray(
            M.transpose(1, 0, 2).reshape(P, TS * P)
        ).astype(ml_dtypes.float8_e4m3)
        stq = np.ascontiguousarray(
            M.transpose(2, 0, 1).reshape(P, TS * P)
        ).astype(ml_dtypes.float8_e4m3)
        dids = (k * NLOC + np.arange(NW)[None, :] * P
                + np.arange(P)[:, None]).astype(np.int32)
        pm = np.zeros((1, NLOC), np.float32)
        for s in (2 * k, 2 * k + 1):
            off = (s - 2 * k) * SEGLEN
            pm[0, off + seg_sizes[s]: off + SEGLEN] = -1e30
        im = {
            "xT": xT, "fcwb": fcwb.astype(np.float32),
            "srcs": idx_tile, "selc": scq, "selt": stq,
            "dstids": np.ascontiguousarray(dids),
            "pmask": pm, "idnt": idnt,
            "pmrep": np.ascontiguousarray(
                np.broadcast_to(pm, (P, NLOC))).astype(ml_dtypes.bfloat16),
            "_didx": dst_slot[k].astype(np.int64).reshape(TS, P).T,
        }
        for l in (1, 2, 3):
            im[f"wc{l}"] = wcs[l - 1]
            im[f"bng{l}"] = bngs[l - 1]
            im[f"bnb{l}"] = bnbs[l - 1]
        in_maps.append(im)
    return SEGLEN, tuple(int(v) for v in nsubs), N, in_maps


_CACHE = {}


def _get_nc(SEGLEN, NSUB, n_real):
    key = (SEGLEN, NSUB, n_real)
    if key not in _CACHE:
        nc = build_gnn(SEGLEN, NSUB, n_real)
        hoist_excess_waits(nc)
        _CACHE[key] = nc
    return _CACHE[key]


def kernel(**inputs):
    patch_walrus()
    SEGLEN, NSUB, n_real, in_maps = prepare(inputs)
    nc = _get_nc(SEGLEN, NSUB, n_real)
    from concourse import bass_utils
    res = bass_utils.run_bass_kernel_spmd(
        nc, in_maps, core_ids=list(range(NCORES)))
    return np.asarray(res.results[0]["out"]).astype(np.float32)
